# revision 2
# baseline (speedup 1.0000x reference)
"""Trainium2 Bass kernel for nn_Net_63754494542044 (v2).

Data-parallel over 8 NeuronCores (8 B-samples each). Host pre-packs
conv1 im2col / conv weights / RoIAlign grid tables; device runs
conv1 -> conv2 -> RoIAlign gather+bilinear -> fc0/emb/red -> 8 GNN rollouts.

v2 changes vs baseline:
- GNN restructured: deg/bias elementwise chains folded into matmuls
  (augmented mask [bm; diag(deg)] injection, rank-1/2 bias matmuls),
  4 relation graphs emitted stage-major for cross-graph parallelism.
- Weight loads moved to the ACT HWDGE ring so im2col loads start at t=0.
- RoIAlign gathers + bilinear interleaved into the conv2 image loop.
"""
import sys
sys.path.insert(0, '/opt/trn_rl_repo')
import numpy as np
from contextlib import ExitStack
import concourse.bass as bass
import concourse.tile as tile
from concourse import mybir
from concourse.bass_utils import run_bass_kernel_spmd

# Walrus wait-slot limits: CTRL-encoded (Drain/NoOp) = 1; others appear
# limited too on this build -- split conservatively.
def split_drain_waits(nc, max_waits=1, max_waits_other=1):
    for fn in nc.m.functions:
        for bb in fn.blocks:
            insts = bb.instructions
            i = 0
            while i < len(insts):
                inst = insts[i]
                si = getattr(inst, 'sync_info', None)
                lim = max_waits if isinstance(inst, (mybir.InstDrain, mybir.InstNoOp)) else max_waits_other
                if si is not None and si.on_wait and len(si.on_wait) > lim:
                    waits = list(si.on_wait)
                    keep = waits[-lim:]
                    extra = waits[:-lim]
                    new_nops = []
                    for k in range(0, len(extra), max_waits):
                        chunk = extra[k:k + max_waits]
                        nop = mybir.InstNoOp(
                            name=nc.get_next_instruction_name(),
                            engine=inst.engine,
                        )
                        nop.sync_info = mybir.SyncInfo(on_wait=chunk, on_update=[])
                        nc.register_instruction(nop)
                        new_nops.append(nop)
                    inst.sync_info = mybir.SyncInfo(on_wait=keep, on_update=list(si.on_update))
                    insts[i:i] = new_nops
                    i += len(new_nops)
                i += 1


B, T, N = 64, 4, 6
IMG, CIN = 128, 3
VE, D, P = 64, 256, 4
SCALE = 0.25
NCORE = 8
BC = B // NCORE          # 8 samples per core
NIMG = BC * T            # 32 images per core
NROI = BC * T * N        # 192 rois per core
NROW = BC * N            # 48 gnn rows per core
NPT = NROI * 16          # 3072 sample points per core
NG = 24                  # gather groups


# ---------------- conv1 im2col (host) ----------------
def conv1_im2col_host(x):  # x [nimg, 3, 128, 128] fp32
    nimg = x.shape[0]
    xp = np.pad(x, ((0, 0), (0, 0), (0, 1), (0, 1)))  # SAME stride2: pad bottom/right only
    cols = np.empty((45, nimg, 64, 32), np.float32)
    k = 0
    for rt in range(3):
        for ct in range(5):
            for ci in range(3):
                # row = 2*oy + rt ; col = 4*j + ct
                cols[k] = xp[:, ci, rt:rt + 127:2, ct:ct + 125:4]
                k += 1
    return cols  # [45, nimg, 64, 32]


def conv1_weights_host(w_conv1):  # [64, 3, 3, 3]
    W2 = np.zeros((45, 128), np.float32)
    for px in range(2):
        for oc in range(64):
            m = px * 64 + oc
            for dy in range(3):
                for dx in range(3):
                    ct = 2 * px + dx
                    assert 0 <= ct <= 4
                    for ci in range(3):
                        W2[(dy * 5 + ct) * 3 + ci, m] = w_conv1[oc, ci, dy, dx]
    return W2


def conv1_host(x, w_conv1, b_conv1):
    cols = conv1_im2col_host(x)          # [45, nimg, 64, 32]
    W2 = conv1_weights_host(w_conv1)     # [45, 128]
    out = np.einsum('kf,kc->cf', cols.reshape(45, -1), W2)  # [128, nimg*64*32]
    out = out.reshape(2, 64, -1, 64, 32)  # [px, oc, img, oy, j]
    feat1 = np.empty((x.shape[0], 64, 64, 64), np.float32)
    feat1[..., 0::2] = np.transpose(out[0], (1, 0, 2, 3))
    feat1[..., 1::2] = np.transpose(out[1], (1, 0, 2, 3))
    feat1 += b_conv1[None, :, None, None]
    return feat1


# ---------------- conv2 weights (host) ----------------
def conv2_weights_host(w_conv2):  # [64, 64, 3, 3]
    Wb = [[np.zeros((128, 128), np.float32) for _ in range(2)] + [np.zeros((64, 128), np.float32)]
          for _ in range(3)]
    for pxo in range(2):
        for oc in range(64):
            m = pxo * 64 + oc
            for dy in range(3):
                for dx in range(3):
                    x_off = 2 * pxo + dx
                    pxi = x_off % 2
                    Xrel = x_off // 2
                    for ci in range(64):
                        if Xrel < 2:
                            Wb[dy][Xrel][pxi * 64 + ci, m] += w_conv2[oc, ci, dy, dx]
                        else:
                            assert pxi == 0
                            Wb[dy][2][ci, m] += w_conv2[oc, ci, dy, dx]
    return Wb


def conv2_host(feat1r, w_conv2, b_conv2):
    nimg = feat1r.shape[0]
    ph = np.zeros((128, nimg, 2, 33, 33), np.float32)
    f = feat1r
    for pxi in range(2):
        for py in range(2):
            ph[pxi * 64:pxi * 64 + 64, :, py, :32, :32] = np.transpose(
                f[:, :, py::2, pxi::2], (1, 0, 2, 3))
    Wb = conv2_weights_host(w_conv2)
    out = np.zeros((128, nimg, 32, 16), np.float32)
    for dy in range(3):
        py, Yoff = dy % 2, dy // 2
        for g in range(3):
            W = Wb[dy][g]
            Ysl = slice(Yoff, Yoff + 32)
            Xidx = g + 2 * np.arange(16)
            rhs = ph[:, :, py, Ysl, :][:, :, :, Xidx]
            if g == 2:
                rhs = rhs[:64]
            out += np.einsum('km,kijx->mijx', W, rhs)
    feat2 = np.empty((nimg, 64, 32, 32), np.float32)
    feat2[..., 0::2] = np.transpose(out[:64], (1, 0, 2, 3))
    feat2[..., 1::2] = np.transpose(out[64:], (1, 0, 2, 3))
    return feat2 + b_conv2[None, :, None, None]


# ---------------- RoIAlign grid (host) ----------------
def roi_grid_host(rois):
    nroi = rois.shape[0]
    W = H = 32
    x1 = rois[:, 1] * SCALE; y1 = rois[:, 2] * SCALE
    x2 = rois[:, 3] * SCALE; y2 = rois[:, 4] * SCALE
    bw = np.maximum(x2 - x1, 1.0) / P
    bh = np.maximum(y2 - y1, 1.0) / P
    grid = np.arange(P, dtype=np.float32) + 0.5
    sx = x1[:, None, None] + bw[:, None, None] * grid[None, None, :]
    sy = y1[:, None, None] + bh[:, None, None] * grid[None, :, None]
    sx = np.broadcast_to(sx, (nroi, P, P)).reshape(-1)
    sy = np.broadcast_to(sy, (nroi, P, P)).reshape(-1)
    x0f = np.clip(np.floor(sx), 0, W - 1)
    y0f = np.clip(np.floor(sy), 0, H - 1)
    lx = np.clip(sx - x0f, 0.0, 1.0)
    ly = np.clip(sy - y0f, 0.0, 1.0)
    x0 = x0f.astype(np.int32); y0 = y0f.astype(np.int32)
    hi = x0 >= 31
    x0 = np.where(hi, 30, x0); lx = np.where(hi, 1.0, lx).astype(np.float32)
    hiy = y0 >= 31
    y0 = np.where(hiy, 30, y0); ly = np.where(hiy, 1.0, ly).astype(np.float32)
    img = np.repeat(np.arange(nroi, dtype=np.int32) // N, 16)
    j2 = x0 >> 1
    par = (x0 & 1).astype(np.float32)
    idx0 = img * 512 + y0 * 16 + j2
    idx1 = idx0 + 16
    w4 = np.stack([(1 - ly) * (1 - lx), (1 - ly) * lx, ly * (1 - lx), ly * lx], 1).astype(np.float32)
    return np.stack([idx0, idx1], 1).astype(np.int32), w4, par


def roi_w6_host(rois):
    idx, w4, par = roi_grid_host(rois)
    wy0 = w4[:, 0] + w4[:, 1]
    wy1 = w4[:, 2] + w4[:, 3]
    lx = np.where(wy0 > 0, w4[:, 1] / np.maximum(wy0, 1e-30), w4[:, 3] / np.maximum(wy1, 1e-30))
    wa = (1 - par) * (1 - lx)
    wb = (1 - par) * lx + par * (1 - lx)
    wc = par * lx
    w6 = np.stack([wy0 * wa, wy0 * wb, wy0 * wc, wy1 * wa, wy1 * wb, wy1 * wc], 1)
    return idx, w6.astype(np.float32)


def feat2_rows_host(feat2r):
    rows = np.transpose(feat2r.reshape(-1, 64, 32, 16, 2), (0, 2, 3, 4, 1)).reshape(-1, 128)
    return np.concatenate([rows, np.zeros((2, 128), rows.dtype)], 0)


def roi_align_host(feat2r, rois):
    idx, w6 = roi_w6_host(rois)
    rows = feat2_rows_host(feat2r)
    g = rows.reshape(-1)
    npt = idx.shape[0]
    blk = np.empty((npt, 2, 256), np.float32)
    for r in range(2):
        for p in range(npt):
            st = idx[p, r] * 128
            blk[p, r] = g[st: st + 256]
    offs = [0, 64, 128, 256, 320, 384]
    b2 = blk.reshape(npt, 512)
    pooled = np.zeros((npt, 64), np.float32)
    for s in range(6):
        pooled += w6[:, s:s + 1] * b2[:, offs[s]: offs[s] + 64]
    return pooled


# ---------------- GNN (host mirror) ----------------
def mask_host(coor, r):
    bm = np.zeros((NROW, NROW), np.float32)
    for b in range(BC):
        d = np.linalg.norm(coor[b][:, None, :] - coor[b][None, :, :], axis=-1)
        m = (d <= (r[b][:, None] + r[b][None, :])) & ~np.eye(N, dtype=bool)
        bm[b * N:(b + 1) * N, b * N:(b + 1) * N] = m
    return bm, bm.sum(1)


def internet_host(s, bm, deg, p):
    sw, sb, rw, rb, aw, ab, ow, ob = p
    Wl, Wr = rw[:, :D], rw[:, D:]
    self_d = s @ sw.T + sb
    u = s @ Wl.T + rb
    v = s @ Wr.T
    rel = deg[:, None] * u + bm @ v
    a = np.maximum((self_d + rel) @ aw.T + ab, 0)
    return np.maximum(a @ ow[:, :D].T + s @ ow[:, D:].T + ob, 0)


def gnn_host(obj_t, src_coor, r, inputs):
    states = list(obj_t)
    masks = [mask_host(src_coor[:, t], r) for t in range(4)]
    num_rollouts = int(inputs['num_rollouts'])
    out = []
    for rr in range(num_rollouts):
        cs = []
        for k in range(4):
            p = (inputs['g_self_w'][k], inputs['g_self_b'][k], inputs['g_rel_w'][k],
                 inputs['g_rel_b'][k], inputs['g_aff_w'][k], inputs['g_aff_b'][k],
                 inputs['g_out_w'][k], inputs['g_out_b'][k])
            bm, deg = masks[k]
            cs.append(internet_host(states[k], bm, deg, p))
        s = np.concatenate(cs, -1) @ inputs['agg_w'].T + inputs['agg_b']
        bbox = s @ inputs['dec_w'].T + inputs['dec_b']
        out.append(bbox.reshape(BC, N, 4))
        states = states[1:] + [s]
        coor = bbox[:, 2:].reshape(BC, N, 2)
        masks = masks[1:] + [mask_host(coor, r)]
    return np.stack(out, 1)


def full_host(inputs, shard):
    sl = slice(shard * BC, (shard + 1) * BC)
    x = inputs['x'][sl].reshape(NIMG, CIN, IMG, IMG)
    rois = inputs['rois'][sl].reshape(NROI, 5)
    coor = inputs['src_coor_features'][sl]
    r = (((rois.reshape(BC, T, N, 5)[..., 4] - rois.reshape(BC, T, N, 5)[..., 2]) / 2
          + (rois.reshape(BC, T, N, 5)[..., 3] - rois.reshape(BC, T, N, 5)[..., 1]) / 2) / 2).mean(1)
    f1 = np.maximum(conv1_host(x, inputs['w_conv1'], inputs['b_conv1']), 0)
    f2 = np.maximum(conv2_host(f1, inputs['w_conv2'], inputs['b_conv2']), 0)
    pooled = roi_align_host(f2, rois)
    pool_cp = pooled.reshape(NROI, 16, 64)
    Wp = inputs['fc0_w'].reshape(D, 64, 16)
    obj = np.einsum('rpc,dcp->rd', pool_cp, Wp) + inputs['fc0_b']
    obj = np.maximum(obj, 0)
    emb = np.maximum(coor.reshape(NROI, 2) @ inputs['fc0c_w'].T + inputs['fc0c_b'], 0)
    emb = np.maximum(emb @ inputs['fc1c_w'].T + inputs['fc1c_b'], 0)
    o2 = np.maximum(obj @ inputs['red_w'][:, :D].T + emb @ inputs['red_w'][:, D:].T
                    + inputs['red_b'], 0)
    o2 = o2.reshape(BC, T, N, D)
    obj_t = [o2[:, t].reshape(NROW, D) for t in range(4)]
    return gnn_host(obj_t, coor, r, inputs)


# ---------------- device input packing ----------------
def make_core_inputs(inputs, shard):
    import ml_dtypes
    bf16 = ml_dtypes.bfloat16
    sl = slice(shard * BC, (shard + 1) * BC)
    x = np.asarray(inputs['x'][sl], np.float32).reshape(NIMG, CIN, IMG, IMG)
    rois = np.asarray(inputs['rois'][sl], np.float32).reshape(NROI, 5)
    coor = np.asarray(inputs['src_coor_features'][sl], np.float32)   # [BC,T,N,2]
    rr5 = rois.reshape(BC, T, N, 5)
    r = (((rr5[..., 4] - rr5[..., 2]) / 2 + (rr5[..., 3] - rr5[..., 1]) / 2) / 2).mean(1)

    d = {}
    cols = conv1_im2col_host(x)                       # [45, NIMG, 64, 32]
    d['im2col45'] = cols.reshape(45, -1).astype(bf16)
    d['w1'] = conv1_weights_host(np.asarray(inputs['w_conv1'])).astype(bf16)
    b1 = np.asarray(inputs['b_conv1'], np.float32)
    d['b1'] = np.tile(b1, 2).reshape(128, 1).astype(np.float32)
    Wb = conv2_weights_host(np.asarray(inputs['w_conv2']))
    d['w2a'] = np.stack([Wb[dy][0] for dy in range(3)]).astype(bf16)
    d['w2b'] = np.stack([Wb[dy][1] for dy in range(3)]).astype(bf16)
    d['w2c'] = np.stack([Wb[dy][2] for dy in range(3)]).astype(bf16)
    b2 = np.asarray(inputs['b_conv2'], np.float32)
    d['b2'] = np.tile(b2, 2).reshape(128, 1).astype(np.float32)

    idx, w6 = roi_w6_host(rois)                       # [NPT,2] int32, [NPT,6]
    d['gidx'] = idx.reshape(NG, 128, 2).transpose(1, 0, 2).reshape(128, NG * 2).copy()
    d['w6'] = w6.reshape(NG, 128, 6).transpose(1, 0, 2).reshape(128, NG * 6).astype(np.float32)

    fc0w = np.asarray(inputs['fc0_w'], np.float32).reshape(D, 64, 16)
    d['fc0t'] = np.ascontiguousarray(fc0w.transpose(2, 1, 0)).astype(bf16)  # [pt, c, d]
    d['fc0b'] = np.asarray(inputs['fc0_b'], np.float32).reshape(2, 128).T.copy()

    d['coor_fm'] = coor.reshape(NROI, 2).T.astype(bf16).copy()

    def t2(w):   # [256, K] -> [kc, 128, 256] lhsT chunks (w.T row-chunks)
        wT = np.ascontiguousarray(np.asarray(w, np.float32).T)       # [K, 256]
        K = wT.shape[0]
        return wT.reshape(K // 128, 128, 256).astype(bf16)

    def bcol(b):  # [256] -> [128, 2]
        return np.asarray(b, np.float32).reshape(2, 128).T.copy()

    d['fc0ct'] = np.asarray(inputs['fc0c_w'], np.float32).T.astype(bf16).copy()  # [2, 256]
    d['fc0cb'] = bcol(inputs['fc0c_b'])
    d['fc1ct'] = t2(inputs['fc1c_w'])
    d['fc1cb'] = bcol(inputs['fc1c_b'])
    redw = np.asarray(inputs['red_w'], np.float32)
    d['redoT'] = t2(redw[:, :D])
    d['redeT'] = t2(redw[:, D:])
    d['redb'] = bcol(inputs['red_b'])

    d['gswT'] = np.stack([t2(inputs['g_self_w'][k]) for k in range(4)])
    d['gawT'] = np.stack([t2(inputs['g_aff_w'][k]) for k in range(4)])
    gow = np.asarray(inputs['g_out_w'], np.float32)
    d['gowaT'] = np.stack([t2(gow[k][:, :D]) for k in range(4)])
    d['gowsT'] = np.stack([t2(gow[k][:, D:]) for k in range(4)])
    d['aggT'] = t2(inputs['agg_w'])                    # [8, 128, 256]
    decw = np.asarray(inputs['dec_w'], np.float32)     # [4, 256]
    d['decT'] = decw.T.reshape(2, 128, 4).astype(bf16).copy()

    # g1T: per (k, kc): [WlT_kc | WrT_kc] -> [128, 4k*2kc*512]
    grw = np.asarray(inputs['g_rel_w'], np.float32)    # [4, 256, 512]
    blocks = []
    for k in range(4):
        Wl = grw[k][:, :D].T                           # [256 d, 256 m]
        Wr = grw[k][:, D:].T
        for kc in range(2):
            blocks.append(np.concatenate(
                [Wl[kc * 128:(kc + 1) * 128], Wr[kc * 128:(kc + 1) * 128]], axis=1))
    d['g1T'] = np.concatenate(blocks, axis=1).astype(bf16)     # [128, 4096]

    # bias rows for rank-1/2 bias matmuls
    d['rbrow'] = np.asarray(inputs['g_rel_b'], np.float32).reshape(1, 4 * 256).astype(bf16)
    d['sb2'] = np.concatenate([np.asarray(inputs['g_self_b'][k], np.float32).reshape(2, 128)
                               for k in range(4)], axis=1).astype(bf16)   # [2, 512]
    d['ab2'] = np.concatenate([np.asarray(inputs['g_aff_b'][k], np.float32).reshape(2, 128)
                               for k in range(4)], axis=1).astype(bf16)
    d['ob2'] = np.concatenate([np.asarray(inputs['g_out_b'][k], np.float32).reshape(2, 128)
                               for k in range(4)], axis=1).astype(bf16)
    d['aggb2'] = np.asarray(inputs['agg_b'], np.float32).reshape(2, 128).astype(bf16)
    d['decbrow'] = np.asarray(inputs['dec_b'], np.float32).reshape(1, 4).astype(bf16)
    blk2 = np.zeros((2, 96), np.float32)
    blk2[0, :48] = 1.0; blk2[1, 48:] = 1.0
    d['blk2'] = blk2.astype(bf16)

    # masks: bm at rows 0:48, diag(deg) at rows 64:112 (partition-base rules)
    hm2 = []
    for m in range(4):
        bm, deg = mask_host(coor[:, m], r)
        hm = np.zeros((128, 48), np.float32)
        hm[0:48] = bm
        hm[64:112] = np.diag(deg)
        hm2.append(hm.astype(bf16))
    d['hm2'] = np.stack(hm2)                           # [4, 128, 48]
    Tmat = np.full((NROW, NROW), -1.0, np.float32)
    for b in range(BC):
        rs = (r[b][:, None] + r[b][None, :]) ** 2
        np.fill_diagonal(rs, -1.0)
        Tmat[b * N:(b + 1) * N, b * N:(b + 1) * N] = rs
    d['Tm'] = Tmat
    d['ones48'] = np.ones((48, 128), bf16)
    d['ones2'] = np.ones((2, 48), bf16)
    d['ident'] = np.eye(128, dtype=bf16)
    return d


dt = mybir.dt
AF = mybir.ActivationFunctionType
OP = mybir.AluOpType

NIMG, NROI, NROW, NPT = 32, 192, 48, 3072
NG = 24            # gather groups (128 pts each)
IMG_GRP = 4        # images per conv group
NGRP = NIMG // IMG_GRP
IMGF = 2 * 33 * 33  # 2178 free els per img in feat1_ph

# gather group gg may be issued after conv2 of image (8*gg+7)//6 + 1
# (+1 because the y0+1 row window of the last position can touch the
#  first row of the next image; weight there is 0 but it must be written)
GATHER_AFTER_IMG = {}
for _gg in range(NG):
    GATHER_AFTER_IMG.setdefault(min((8 * _gg + 7) // 6 + 1, NIMG - 1), []).append(_gg)


def build(nc: bass.Bass, zero_bias=False):
    f32, bf16, i32 = dt.float32, dt.bfloat16, dt.int32

    def din(name, shape, d):
        return nc.dram_tensor(name, shape, d, kind="ExternalInput")

    im2col = din("im2col45", [45, 65536], bf16)
    w1 = din("w1", [45, 128], bf16)
    b1 = din("b1", [128, 1], f32)
    w2a = din("w2a", [3, 128, 128], bf16)
    w2b = din("w2b", [3, 128, 128], bf16)
    w2c = din("w2c", [3, 64, 128], bf16)
    b2 = din("b2", [128, 1], f32)
    gidx = din("gidx", [128, 48], i32)
    w6 = din("w6", [128, 144], f32)
    fc0t = din("fc0t", [16, 64, 256], bf16)
    fc0b = din("fc0b", [128, 2], f32)
    coor = din("coor_fm", [2, 192], bf16)
    fc0ct = din("fc0ct", [2, 256], bf16)
    fc0cb = din("fc0cb", [128, 2], f32)
    fc1ct = din("fc1ct", [2, 128, 256], bf16)
    fc1cb = din("fc1cb", [128, 2], f32)
    redoT = din("redoT", [2, 128, 256], bf16)
    redeT = din("redeT", [2, 128, 256], bf16)
    redb = din("redb", [128, 2], f32)
    gswT = din("gswT", [4, 2, 128, 256], bf16)
    gawT = din("gawT", [4, 2, 128, 256], bf16)
    gowaT = din("gowaT", [4, 2, 128, 256], bf16)
    gowsT = din("gowsT", [4, 2, 128, 256], bf16)
    g1T = din("g1T", [128, 4096], bf16)
    rbrow = din("rbrow", [1, 1024], bf16)
    sb2 = din("sb2", [2, 512], bf16)
    ab2 = din("ab2", [2, 512], bf16)
    ob2 = din("ob2", [2, 512], bf16)
    aggb2 = din("aggb2", [2, 128], bf16)
    decbrow = din("decbrow", [1, 4], bf16)
    blk2 = din("blk2", [2, 96], bf16)
    aggT = din("aggT", [8, 128, 256], bf16)
    decT = din("decT", [2, 128, 4], bf16)
    hm2 = din("hm2", [4, 128, 48], bf16)
    Tm = din("Tm", [48, 48], f32)
    ones48 = din("ones48", [48, 128], bf16)
    ones2 = din("ones2", [2, 48], bf16)
    ident = din("ident", [128, 128], bf16)

    out = nc.dram_tensor("bbox_out", [8, 8, 6, 4], f32, kind="ExternalOutput")

    with tile.TileContext(nc) as tc, ExitStack() as ctx:
        # ---- persistent pools ----
        wp = ctx.enter_context(tc.tile_pool(name="w", bufs=1))
        dramp = ctx.enter_context(tc.tile_pool(name="dram", bufs=1, space="DRAM"))
        sp = ctx.enter_context(tc.tile_pool(name="state", bufs=1))

        # conv-critical weights on the SP ring (small, before im2col loads)
        def loads(dram_t, shape, dtype, src_ap=None, eng=None):
            t = wp.tile(shape, dtype, tag=dram_t.name)
            e = eng if eng is not None else nc.sync
            if src_ap is None:
                e.dma_start(t[:], dram_t[:, :])
            else:
                dims = [c for _, c in src_ap.ap[1:]]
                spec = " ".join(f"d{i}" for i in range(len(dims)))
                kw = {f"d{i}": dims[i] for i in range(len(dims) - 1)}
                dv = t[:].rearrange(f"p ({spec}) -> p {spec}", **kw)
                e.dma_start(dv, src_ap)
            return t

        w1_s = loads(w1, [45, 128], bf16)
        b1_s = loads(b1, [128, 1], f32)
        w2a_s = loads(w2a, [128, 3 * 128], bf16, w2a[:].rearrange("d p m -> p d m"))
        w2b_s = loads(w2b, [128, 3 * 128], bf16, w2b[:].rearrange("d p m -> p d m"))
        w2c_s = loads(w2c, [64, 3 * 128], bf16, w2c[:].rearrange("d p m -> p d m"))
        b2_s = loads(b2, [128, 1], f32)
        ident_s = loads(ident, [128, 128], bf16)

        # everything else on the ACT HWDGE ring (overlaps im2col loads)
        def load(dram_t, shape, dtype, src_ap=None):
            return loads(dram_t, shape, dtype, src_ap, eng=nc.scalar)

        gidx_s = load(gidx, [128, 48], i32)
        w6_s = load(w6, [128, 144], f32)
        fc0t_s = load(fc0t, [64, 16 * 256], bf16, fc0t[:].rearrange("t p m -> p t m"))
        fc0b_s = load(fc0b, [128, 2], f32)
        coor_s = load(coor, [2, 192], bf16)
        fc0ct_s = load(fc0ct, [2, 256], bf16)
        fc0cb_s = load(fc0cb, [128, 2], f32)
        fc1ct_s = load(fc1ct, [128, 512], bf16, fc1ct[:].rearrange("k p m -> p k m"))
        fc1cb_s = load(fc1cb, [128, 2], f32)
        redoT_s = load(redoT, [128, 512], bf16, redoT[:].rearrange("k p m -> p k m"))
        redeT_s = load(redeT, [128, 512], bf16, redeT[:].rearrange("k p m -> p k m"))
        redb_s = load(redb, [128, 2], f32)

        def loadg(t):  # [4,2,128,256] -> [128, 4*512]
            return load(t, [128, 2048], bf16, t[:].rearrange("h k p m -> p h k m"))
        gswT_s = loadg(gswT)
        gawT_s, gowaT_s, gowsT_s = loadg(gawT), loadg(gowaT), loadg(gowsT)
        g1T_s = load(g1T, [128, 4096], bf16)
        rbrow_s = load(rbrow, [1, 1024], bf16)
        sb2_s = load(sb2, [2, 512], bf16)
        ab2_s = load(ab2, [2, 512], bf16)
        ob2_s = load(ob2, [2, 512], bf16)
        aggb2_s = load(aggb2, [2, 128], bf16)
        decbrow_s = load(decbrow, [1, 4], bf16)
        blk2_s = load(blk2, [2, 96], bf16)
        aggT_s = load(aggT, [128, 2048], bf16, aggT[:].rearrange("k p m -> p k m"))
        decT_s = load(decT, [128, 8], bf16, decT[:].rearrange("k p m -> p k m"))
        Tm_s = load(Tm, [48, 48], f32)
        ones48_s = load(ones48, [48, 128], bf16)
        ones2_s = load(ones2, [2, 48], bf16)

        # mask2 slots: bm rows 0:48, diag(deg) rows 64:112 (0..3 from host)
        mask2_t = [sp.tile([128, 48], bf16, name=f"m2_{m}", tag=f"m2_{m}") for m in range(11)]
        vu_t = [sp.tile([128, 256], bf16, name=f"vu{k}", tag=f"vu{k}") for k in range(4)]
        for m in range(11):
            nc.gpsimd.memset(mask2_t[m][32:64, :], 0.0)
        for k in range(4):
            nc.gpsimd.memset(vu_t[k][32:64, :], 0.0)
        for m in range(4):
            nc.scalar.dma_start(mask2_t[m][:], hm2[m])

        st = [sp.tile([128, 96], bf16, name=f"st{m}", tag=f"st{m}") for m in range(12)]
        bbox_sb = sp.tile([4, 384], f32, tag="bbox")
        poolT = sp.tile([64, 3072], bf16, tag="poolT")
        fd = dramp.tile([16386, 128], bf16, tag="feat2")
        zpad = sp.tile([2, 128], bf16, tag="zpad")
        nc.gpsimd.memset(zpad[:], 0.0)
        nc.gpsimd.dma_start(fd[16384:16386, :], zpad[:])

        # ================= conv stage (+ interleaved roi gather) =============
        with ExitStack() as cvx:
            imcp = cvx.enter_context(tc.tile_pool(name="imc", bufs=2))
            f1p = cvx.enter_context(tc.tile_pool(name="f1", bufs=2))
            c1ps = cvx.enter_context(tc.tile_pool(name="c1ps", bufs=2, space="PSUM"))
            c2ps = cvx.enter_context(tc.tile_pool(name="c2ps", bufs=2, space="PSUM"))
            tps = cvx.enter_context(tc.tile_pool(name="tps", bufs=1, space="PSUM"))
            f2p = cvx.enter_context(tc.tile_pool(name="f2", bufs=3))
            gp = cvx.enter_context(tc.tile_pool(name="g", bufs=3))
            bp = cvx.enter_context(tc.tile_pool(name="bil", bufs=3))
            ptps = cvx.enter_context(tc.tile_pool(name="ptps", bufs=1, space="PSUM"))

            def roi_gather_group(g):
                gb = gp.tile([128, 512], bf16, tag="gb")
                for rrow in range(2):
                    nc.gpsimd.indirect_dma_start(
                        out=gb[:, rrow * 256:(rrow + 1) * 256], out_offset=None, in_=fd[:],
                        in_offset=bass.IndirectOffsetOnAxis(
                            ap=gidx_s[:, 2 * g + rrow:2 * g + rrow + 1], axis=0))
                offs = [0, 64, 128, 256, 320, 384]
                a0 = bp.tile([128, 64], f32, tag="acc0")
                a1 = bp.tile([128, 64], f32, tag="acc1")
                nc.vector.tensor_scalar(out=a0[:], in0=gb[:, 0:64],
                                        scalar1=w6_s[:, 6 * g:6 * g + 1], scalar2=None, op0=OP.mult)
                cur, nxt = a0, a1
                dst = None
                for s in range(1, 6):
                    dst = bp.tile([128, 64], bf16, name="pb", tag="pb") if s == 5 else nxt
                    nc.vector.scalar_tensor_tensor(
                        out=dst[:], in0=gb[:, offs[s]:offs[s] + 64],
                        scalar=w6_s[:, 6 * g + s:6 * g + s + 1], in1=cur[:],
                        op0=OP.mult, op1=OP.add)
                    if s < 5:
                        cur, nxt = dst, cur
                pb = dst
                pt = ptps.tile([64, 128], bf16, tag="pt")
                nc.tensor.transpose(pt[:], pb[:], ident_s[:])
                nc.scalar.activation(out=poolT[:, 128 * g:128 * (g + 1)], in_=pt[:], func=AF.Copy)

            for g in range(NGRP):
                imc = imcp.tile([45, IMG_GRP * 2048], bf16, tag="imc")
                nc.sync.dma_start(imc[:], im2col[:, g * IMG_GRP * 2048:
                                                   (g + 1) * IMG_GRP * 2048])
                f1 = f1p.tile([128, IMG_GRP * IMGF], bf16, tag="f1")
                f1v = f1[:].rearrange("p (i y x) -> p i y x", i=IMG_GRP, y=2 * 33, x=33)
                nc.gpsimd.memset(f1v[:, :, :, 32:33], 0.0)
                f1h = f1[:].rearrange("p (i py y x) -> p i py y x", i=IMG_GRP, py=2, y=33, x=33)
                nc.gpsimd.memset(f1h[:, :, :, 32:33, :], 0.0)
                for i in range(IMG_GRP):
                    pv = []
                    for h in range(2):
                        ps = c1ps.tile([128, 1024], f32, tag="c1")
                        for q in range(2):
                            nc.tensor.matmul(ps[:, q * 512:(q + 1) * 512], lhsT=w1_s[:],
                                             rhs=imc[:, i * 2048 + h * 1024 + q * 512:
                                                     i * 2048 + h * 1024 + (q + 1) * 512],
                                             start=True, stop=True)
                        pv.append(ps)
                    for h in range(2):
                        psv = pv[h][:].rearrange("p (y j) -> p y j", y=32)
                        for py in range(2):
                            dst = f1h[:, i, py, 16 * h:16 * h + 16, 0:32]
                            if py:
                                nc.vector.tensor_scalar(
                                    out=dst, in0=psv[:, py::2, :], scalar1=b1_s[:, 0:1],
                                    scalar2=0.0, op0=OP.add, op1=OP.max)
                            else:
                                nc.scalar.activation(out=dst, in_=psv[:, py::2, :],
                                                     func=AF.Relu, bias=b1_s[:, 0:1])
                for i in range(IMG_GRP):
                    ps = c2ps.tile([128, 512], f32, tag="c2")
                    first = True
                    f1v5 = f1[:].rearrange("p (i py y x) -> p i py y x",
                                           i=IMG_GRP, py=2, y=33, x=33)
                    for dy in range(3):
                        py, yo = dy % 2, dy // 2
                        for gsel in range(3):
                            sl = f1v5[:, i, py, yo:yo + 32, gsel:gsel + 1]
                            rhs_ap = bass.AP(sl.tensor, sl.offset,
                                             [sl.ap[0], sl.ap[1], [2, 16]])
                            if gsel == 2:
                                rhs_ap = rhs_ap[0:64]
                                lhsT = w2c_s[:, dy * 128:(dy + 1) * 128]
                            else:
                                lhsT = (w2a_s if gsel == 0 else w2b_s)[:, dy * 128:(dy + 1) * 128]
                            nc.tensor.matmul(ps[:], lhsT=lhsT, rhs=rhs_ap,
                                             start=first, stop=(dy == 2 and gsel == 2))
                            first = False
                    f2s = f2p.tile([128, 512], bf16, tag="f2s")
                    if i % 2 == 0:
                        nc.vector.tensor_scalar(out=f2s[:], in0=ps[:], scalar1=b2_s[:, 0:1],
                                                scalar2=0.0, op0=OP.add, op1=OP.max)
                    else:
                        nc.scalar.activation(out=f2s[:], in_=ps[:], func=AF.Relu,
                                             bias=b2_s[:, 0:1])
                    tp = tps.tile([128, 512], bf16, tag="tp")
                    for b in range(4):
                        nc.tensor.transpose(tp[:, b * 128:(b + 1) * 128],
                                            f2s[:, b * 128:(b + 1) * 128], ident_s[:])
                    f2t = f2p.tile([128, 512], bf16, tag="f2t")
                    if i % 2 == 0:
                        nc.scalar.activation(out=f2t[:], in_=tp[:], func=AF.Copy)
                    else:
                        nc.vector.tensor_copy(out=f2t[:], in_=tp[:])
                    img = g * IMG_GRP + i
                    dst = fd[img * 512:(img + 1) * 512, :].rearrange(
                        "(b p) c -> p b c", p=128)
                    nc.sync.dma_start(dst, f2t[:].rearrange("p (b c) -> p b c", c=128))
                    for gg in GATHER_AFTER_IMG.get(img, []):
                        roi_gather_group(gg)

        # ================= fc0 + emb + red =================
        with ExitStack() as gx:
            ops = gx.enter_context(tc.tile_pool(name="ops", bufs=2, space="PSUM"))

            obj = sp.tile([128, 384], bf16, tag="obj")
            pview = poolT[:].rearrange("p (r t) -> p t r", t=16)
            for m2 in range(2):
                ps = ops.tile([128, 192], f32, tag="obj")
                for pt_i in range(16):
                    nc.tensor.matmul(ps[:], lhsT=fc0t_s[:, pt_i * 256 + m2 * 128:
                                                        pt_i * 256 + m2 * 128 + 128],
                                     rhs=pview[:, pt_i, :],
                                     start=(pt_i == 0), stop=(pt_i == 15))
                nc.scalar.activation(out=obj[:, m2 * 192:(m2 + 1) * 192], in_=ps[:],
                                     func=AF.Relu, bias=fc0b_s[:, m2:m2 + 1])
            emb1 = sp.tile([128, 384], bf16, tag="emb1")
            for m2 in range(2):
                ps = ops.tile([128, 192], f32, tag="emb")
                nc.tensor.matmul(ps[:], lhsT=fc0ct_s[:, m2 * 128:(m2 + 1) * 128],
                                 rhs=coor_s[:], start=True, stop=True)
                nc.scalar.activation(out=emb1[:, m2 * 192:(m2 + 1) * 192], in_=ps[:],
                                     func=AF.Relu, bias=fc0cb_s[:, m2:m2 + 1])
            emb2 = sp.tile([128, 384], bf16, tag="emb2")
            for m2 in range(2):
                ps = ops.tile([128, 192], f32, tag="emb")
                for kc in range(2):
                    nc.tensor.matmul(ps[:], lhsT=fc1ct_s[:, kc * 256 + m2 * 128:
                                                         kc * 256 + m2 * 128 + 128],
                                     rhs=emb1[:, kc * 192:(kc + 1) * 192],
                                     start=(kc == 0), stop=(kc == 1))
                nc.scalar.activation(out=emb2[:, m2 * 192:(m2 + 1) * 192], in_=ps[:],
                                     func=AF.Relu, bias=fc1cb_s[:, m2:m2 + 1])
            o2 = sp.tile([128, 384], bf16, tag="o2")
            for m2 in range(2):
                ps = ops.tile([128, 192], f32, tag="o2")
                for kc in range(2):
                    nc.tensor.matmul(ps[:], lhsT=redoT_s[:, kc * 256 + m2 * 128:
                                                         kc * 256 + m2 * 128 + 128],
                                     rhs=obj[:, kc * 192:(kc + 1) * 192],
                                     start=(kc == 0), stop=False)
                for kc in range(2):
                    nc.tensor.matmul(ps[:], lhsT=redeT_s[:, kc * 256 + m2 * 128:
                                                         kc * 256 + m2 * 128 + 128],
                                     rhs=emb2[:, kc * 192:(kc + 1) * 192],
                                     start=False, stop=(kc == 1))
                nc.scalar.activation(out=o2[:, m2 * 192:(m2 + 1) * 192], in_=ps[:],
                                     func=AF.Relu, bias=redb_s[:, m2:m2 + 1])
            o2v = o2[:].rearrange("p (m2 b t n) -> p m2 b t n", m2=2, b=8, t=4)
            for m in range(4):
                nc.vector.tensor_copy(
                    out=st[m][:].rearrange("p (m2 b n) -> p m2 b n", m2=2, b=8),
                    in_=o2v[:, :, :, m, :])

        # ================= GNN rollouts (v2) =================
        with ExitStack() as rx:
            p1p = rx.enter_context(tc.tile_pool(name="p1p", bufs=2, space="PSUM"))
            pxp = rx.enter_context(tc.tile_pool(name="pxp", bufs=1, space="PSUM"))
            msp = rx.enter_context(tc.tile_pool(name="msp", bufs=1, space="PSUM"))
            hb = rx.enter_context(tc.tile_pool(name="hbuf", bufs=4))
            cb = rx.enter_context(tc.tile_pool(name="cbuf", bufs=5))

            onesrow = ones2_s[0:1, :]

            for rr in range(8):
                ps1 = {}
                vu = {}
                xs = {}
                asb = {}
                cs = {}
                # --- step1: u' | v  (row-major, stationary = state chunks) ---
                for k in range(4):
                    m = rr + k
                    s = st[m]
                    p1 = p1p.tile([48, 512], f32, tag="p1")
                    nc.tensor.matmul(p1[:], lhsT=s[:, 0:48],
                                     rhs=g1T_s[:, k * 1024:k * 1024 + 512],
                                     start=True, stop=False)
                    if not zero_bias:
                        nc.tensor.matmul(p1[:, 0:256], lhsT=onesrow,
                                         rhs=rbrow_s[:, k * 256:(k + 1) * 256],
                                         start=False, stop=False)
                    nc.tensor.matmul(p1[:], lhsT=s[:, 48:96],
                                     rhs=g1T_s[:, k * 1024 + 512:(k + 1) * 1024],
                                     start=False, stop=True)
                    ps1[k] = p1
                # --- evac1: vu = v rows 0:48, u' rows 64:112, bf16 ---
                for k in range(4):
                    t = vu_t[k]
                    nc.vector.tensor_copy(out=t[0:48, :], in_=ps1[k][:, 256:512])
                    nc.scalar.activation(out=t[64:112, :], in_=ps1[k][:, 0:256], func=AF.Copy)
                    vu[k] = t
                # --- psum_x: self + mask2-injection + bias ---
                pxt = pxp.tile([128, 384], f32, tag="px")
                for k in range(4):
                    m = rr + k
                    s = st[m]
                    px = pxt[:, k * 96:(k + 1) * 96]
                    for m2 in range(2):
                        for kc in range(2):
                            lo = k * 512 + kc * 256 + m2 * 128
                            nc.tensor.matmul(px[:, m2 * 48:m2 * 48 + 48],
                                             lhsT=gswT_s[:, lo:lo + 128],
                                             rhs=s[:, kc * 48:kc * 48 + 48],
                                             start=(k == 0 and m2 == 0 and kc == 0),
                                             stop=False)
                    for m2 in range(2):
                        nc.tensor.matmul(px[:, m2 * 48:m2 * 48 + 48],
                                         lhsT=vu[k][0:112, m2 * 128:(m2 + 1) * 128],
                                         rhs=mask2_t[m][0:112, :],
                                         start=False,
                                         stop=(zero_bias and k == 3 and m2 == 1))
                    if not zero_bias:
                        nc.tensor.matmul(px[:], lhsT=sb2_s[:, k * 128:(k + 1) * 128],
                                         rhs=blk2_s[:], start=False, stop=(k == 3))
                    xs[k] = px
                # --- x evac ---
                for k in range(4):
                    t = hb.tile([128, 96], bf16, tag="xsb")
                    nc.vector.tensor_copy(out=t[:], in_=xs[k][:])
                    xs[k] = t
                # --- aff ---
                pat = pxp.tile([128, 384], f32, tag="pa")
                for k in range(4):
                    pa = pat[:, k * 96:(k + 1) * 96]
                    for m2 in range(2):
                        for kc in range(2):
                            lo = k * 512 + kc * 256 + m2 * 128
                            nc.tensor.matmul(pa[:, m2 * 48:m2 * 48 + 48],
                                             lhsT=gawT_s[:, lo:lo + 128],
                                             rhs=xs[k][:, kc * 48:kc * 48 + 48],
                                             start=(k == 0 and m2 == 0 and kc == 0),
                                             stop=(zero_bias and k == 3
                                                   and m2 == 1 and kc == 1))
                    if not zero_bias:
                        nc.tensor.matmul(pa[:], lhsT=ab2_s[:, k * 128:(k + 1) * 128],
                                         rhs=blk2_s[:], start=False, stop=(k == 3))
                    asb[k] = pa
                for k in range(4):
                    t = hb.tile([128, 96], bf16, tag="asb")
                    nc.scalar.activation(out=t[:], in_=asb[k][:], func=AF.Relu)
                    asb[k] = t
                # --- out ---
                pot = pxp.tile([128, 384], f32, tag="po")
                for k in range(4):
                    m = rr + k
                    s = st[m]
                    po = pot[:, k * 96:(k + 1) * 96]
                    for m2 in range(2):
                        for kc in range(2):
                            lo = k * 512 + kc * 256 + m2 * 128
                            nc.tensor.matmul(po[:, m2 * 48:m2 * 48 + 48],
                                             lhsT=gowaT_s[:, lo:lo + 128],
                                             rhs=asb[k][:, kc * 48:kc * 48 + 48],
                                             start=(k == 0 and m2 == 0 and kc == 0),
                                             stop=False)
                            nc.tensor.matmul(po[:, m2 * 48:m2 * 48 + 48],
                                             lhsT=gowsT_s[:, lo:lo + 128],
                                             rhs=s[:, kc * 48:kc * 48 + 48],
                                             start=False,
                                             stop=(zero_bias and k == 3
                                                   and kc == 1 and m2 == 1))
                    if not zero_bias:
                        nc.tensor.matmul(po[:], lhsT=ob2_s[:, k * 128:(k + 1) * 128],
                                         rhs=blk2_s[:], start=False, stop=(k == 3))
                    cs[k] = po
                for k in range(4):
                    t = cb.tile([128, 96], bf16, tag=f"csb{k}")
                    nc.scalar.activation(out=t[:], in_=cs[k][:], func=AF.Relu)
                    cs[k] = t
                # --- agg ---
                g_ps = pxp.tile([128, 96], f32, tag="g")
                for m2 in range(2):
                    n = 0
                    for k in range(4):
                        for kc in range(2):
                            lo = (k * 2 + kc) * 256 + m2 * 128
                            nc.tensor.matmul(g_ps[:, m2 * 48:m2 * 48 + 48],
                                             lhsT=aggT_s[:, lo:lo + 128],
                                             rhs=cs[k][:, kc * 48:kc * 48 + 48],
                                             start=(m2 == 0 and n == 0),
                                             stop=(zero_bias and m2 == 1 and n == 7))
                            n += 1
                if not zero_bias:
                    nc.tensor.matmul(g_ps[:], lhsT=aggb2_s[:], rhs=blk2_s[:],
                                     start=False, stop=True)
                s_new = st[rr + 4]
                nc.vector.tensor_copy(out=s_new[:], in_=g_ps[:])
                # --- dec ---
                ms = msp.tile([48, 144], f32, tag="ms")
                d_ps = ms[0:4, 0:48]
                for kc in range(2):
                    nc.tensor.matmul(d_ps[:], lhsT=decT_s[:, kc * 4:kc * 4 + 4],
                                     rhs=s_new[:, kc * 48:kc * 48 + 48],
                                     start=(kc == 0), stop=zero_bias and kc == 1)
                if not zero_bias:
                    nc.tensor.matmul(d_ps[:], lhsT=decbrow_s[:], rhs=onesrow,
                                     start=False, stop=True)
                bbv = bbox_sb[:].rearrange("f (b q) -> f b q", b=8)[:, :, rr * 6:rr * 6 + 6]
                nc.vector.tensor_copy(out=bbv, in_=d_ps[:])
                # --- mask for slot rr+4 ---
                if rr < 7:
                    m = rr + 4
                    d2_ps = ms[0:2, 48:96]
                    for kc in range(2):
                        nc.tensor.matmul(d2_ps[:], lhsT=decT_s[:, kc * 4 + 2:kc * 4 + 4],
                                         rhs=s_new[:, kc * 48:kc * 48 + 48],
                                         start=(kc == 0), stop=zero_bias and kc == 1)
                    if not zero_bias:
                        nc.tensor.matmul(d2_ps[:], lhsT=decbrow_s[:, 2:4], rhs=onesrow,
                                         start=False, stop=True)
                    coorb = hb.tile([2, 48], bf16, tag="coorb")
                    nc.vector.tensor_copy(out=coorb[:], in_=d2_ps[:])
                    cm2 = hb.tile([2, 48], bf16, tag="cm2")
                    nc.vector.tensor_scalar(out=cm2[:], in0=coorb[:], scalar1=-2.0,
                                            scalar2=None, op0=OP.mult)
                    sq = hb.tile([2, 48], bf16, tag="sq")
                    nc.vector.tensor_tensor(out=sq[:], in0=coorb[:], in1=coorb[:], op=OP.mult)
                    m_ps = ms[0:48, 96:144]
                    nc.tensor.matmul(m_ps[:], lhsT=coorb[:], rhs=cm2[:], start=True, stop=False)
                    nc.tensor.matmul(m_ps[:], lhsT=sq[:], rhs=ones2_s[:], start=False, stop=False)
                    nc.tensor.matmul(m_ps[:], lhsT=ones2_s[:], rhs=sq[:], start=False, stop=True)
                    nc.vector.tensor_tensor(out=mask2_t[m][0:48, :], in0=m_ps[:], in1=Tm_s[:],
                                            op=OP.is_le)
                    degc = hb.tile([48, 1], f32, tag="degc")
                    nc.vector.tensor_reduce(out=degc[:], in_=mask2_t[m][0:48, :],
                                            axis=mybir.AxisListType.X, op=OP.add)
                    nc.vector.tensor_scalar(out=mask2_t[m][64:112, :],
                                            in0=ident_s[0:48, 0:48],
                                            scalar1=degc[:, 0:1], scalar2=None,
                                            op0=OP.mult)
            nc.sync.dma_start(
                out[:].rearrange("b rr n f -> f (b rr n)"), bbox_sb[:])
    return nc


_NC = None
_NC_ZB = None


def _get_nc(zero_bias=False):
    global _NC, _NC_ZB
    if _NC is None or _NC_ZB != zero_bias:
        nc = bass.Bass()
        build(nc, zero_bias=zero_bias)
        split_drain_waits(nc)
        _NC = nc
        _NC_ZB = zero_bias
    return _NC


def _biases_zero(inputs):
    names = ['b_conv1', 'b_conv2', 'fc0_b', 'fc0c_b', 'fc1c_b', 'red_b',
             'g_self_b', 'g_rel_b', 'g_aff_b', 'g_out_b', 'agg_b', 'dec_b']
    return all(not np.any(np.asarray(inputs[n])) for n in names)


def kernel(**inputs):
    inputs = {k: np.asarray(v) for k, v in inputs.items()}
    nc = _get_nc(zero_bias=_biases_zero(inputs))
    maps = [make_core_inputs(inputs, s) for s in range(NCORE)]
    res = run_bass_kernel_spmd(nc, maps, core_ids=list(range(NCORE)))
    out = np.concatenate([res.results[s]["bbox_out"] for s in range(NCORE)], 0)
    return out.astype(np.float32)


# revision 5
# speedup vs baseline: 1.6222x; 1.6222x over previous
"""Trainium2 Bass kernel for nn_Net_63754494542044 (v2).

Data-parallel over 8 NeuronCores (8 B-samples each). Host pre-packs
conv1 im2col / conv weights / RoIAlign grid tables; device runs
conv1 -> conv2 -> RoIAlign gather+bilinear -> fc0/emb/red -> 8 GNN rollouts.

v2 changes vs baseline:
- GNN restructured: deg/bias elementwise chains folded into matmuls
  (augmented mask [bm; diag(deg)] injection, rank-1/2 bias matmuls),
  4 relation graphs emitted stage-major for cross-graph parallelism.
- Weight loads moved to the ACT HWDGE ring so im2col loads start at t=0.
- RoIAlign gathers + bilinear interleaved into the conv2 image loop.
"""
import sys
sys.path.insert(0, '/opt/trn_rl_repo')
import numpy as np
from contextlib import ExitStack
import concourse.bass as bass
import concourse.tile as tile
from concourse import mybir
from concourse.bass_utils import run_bass_kernel_spmd

# Walrus wait-slot limits: CTRL-encoded (Drain/NoOp) = 1; others appear
# limited too on this build -- split conservatively.
def split_drain_waits(nc, max_waits=1, max_waits_other=1):
    for fn in nc.m.functions:
        for bb in fn.blocks:
            insts = bb.instructions
            i = 0
            while i < len(insts):
                inst = insts[i]
                si = getattr(inst, 'sync_info', None)
                lim = max_waits if isinstance(inst, (mybir.InstDrain, mybir.InstNoOp)) else max_waits_other
                if si is not None and si.on_wait and len(si.on_wait) > lim:
                    waits = list(si.on_wait)
                    keep = waits[-lim:]
                    extra = waits[:-lim]
                    new_nops = []
                    for k in range(0, len(extra), max_waits):
                        chunk = extra[k:k + max_waits]
                        nop = mybir.InstNoOp(
                            name=nc.get_next_instruction_name(),
                            engine=inst.engine,
                        )
                        nop.sync_info = mybir.SyncInfo(on_wait=chunk, on_update=[])
                        nc.register_instruction(nop)
                        new_nops.append(nop)
                    inst.sync_info = mybir.SyncInfo(on_wait=keep, on_update=list(si.on_update))
                    insts[i:i] = new_nops
                    i += len(new_nops)
                i += 1


B, T, N = 64, 4, 6
IMG, CIN = 128, 3
VE, D, P = 64, 256, 4
SCALE = 0.25
NCORE = 8
BC = B // NCORE          # 8 samples per core
NIMG = BC * T            # 32 images per core
NROI = BC * T * N        # 192 rois per core
NROW = BC * N            # 48 gnn rows per core
NPT = NROI * 16          # 3072 sample points per core
NG = 24                  # gather groups


# ---------------- conv1 im2col (host) ----------------
def conv1_im2col_host(x):  # x [nimg, 3, 128, 128] fp32
    nimg = x.shape[0]
    xp = np.pad(x, ((0, 0), (0, 0), (0, 1), (0, 1)))  # SAME stride2: pad bottom/right only
    cols = np.empty((45, nimg, 64, 32), np.float32)
    k = 0
    for rt in range(3):
        for ct in range(5):
            for ci in range(3):
                # row = 2*oy + rt ; col = 4*j + ct
                cols[k] = xp[:, ci, rt:rt + 127:2, ct:ct + 125:4]
                k += 1
    return cols  # [45, nimg, 64, 32]


def conv1_weights_host(w_conv1):  # [64, 3, 3, 3]
    W2 = np.zeros((45, 128), np.float32)
    for px in range(2):
        for oc in range(64):
            m = px * 64 + oc
            for dy in range(3):
                for dx in range(3):
                    ct = 2 * px + dx
                    assert 0 <= ct <= 4
                    for ci in range(3):
                        W2[(dy * 5 + ct) * 3 + ci, m] = w_conv1[oc, ci, dy, dx]
    return W2


def conv1_host(x, w_conv1, b_conv1):
    cols = conv1_im2col_host(x)          # [45, nimg, 64, 32]
    W2 = conv1_weights_host(w_conv1)     # [45, 128]
    out = np.einsum('kf,kc->cf', cols.reshape(45, -1), W2)  # [128, nimg*64*32]
    out = out.reshape(2, 64, -1, 64, 32)  # [px, oc, img, oy, j]
    feat1 = np.empty((x.shape[0], 64, 64, 64), np.float32)
    feat1[..., 0::2] = np.transpose(out[0], (1, 0, 2, 3))
    feat1[..., 1::2] = np.transpose(out[1], (1, 0, 2, 3))
    feat1 += b_conv1[None, :, None, None]
    return feat1


# ---------------- conv2 weights (host) ----------------
def conv2_weights_host(w_conv2):  # [64, 64, 3, 3]
    Wb = [[np.zeros((128, 128), np.float32) for _ in range(2)] + [np.zeros((64, 128), np.float32)]
          for _ in range(3)]
    for pxo in range(2):
        for oc in range(64):
            m = pxo * 64 + oc
            for dy in range(3):
                for dx in range(3):
                    x_off = 2 * pxo + dx
                    pxi = x_off % 2
                    Xrel = x_off // 2
                    for ci in range(64):
                        if Xrel < 2:
                            Wb[dy][Xrel][pxi * 64 + ci, m] += w_conv2[oc, ci, dy, dx]
                        else:
                            assert pxi == 0
                            Wb[dy][2][ci, m] += w_conv2[oc, ci, dy, dx]
    return Wb


def conv2_host(feat1r, w_conv2, b_conv2):
    nimg = feat1r.shape[0]
    ph = np.zeros((128, nimg, 2, 33, 33), np.float32)
    f = feat1r
    for pxi in range(2):
        for py in range(2):
            ph[pxi * 64:pxi * 64 + 64, :, py, :32, :32] = np.transpose(
                f[:, :, py::2, pxi::2], (1, 0, 2, 3))
    Wb = conv2_weights_host(w_conv2)
    out = np.zeros((128, nimg, 32, 16), np.float32)
    for dy in range(3):
        py, Yoff = dy % 2, dy // 2
        for g in range(3):
            W = Wb[dy][g]
            Ysl = slice(Yoff, Yoff + 32)
            Xidx = g + 2 * np.arange(16)
            rhs = ph[:, :, py, Ysl, :][:, :, :, Xidx]
            if g == 2:
                rhs = rhs[:64]
            out += np.einsum('km,kijx->mijx', W, rhs)
    feat2 = np.empty((nimg, 64, 32, 32), np.float32)
    feat2[..., 0::2] = np.transpose(out[:64], (1, 0, 2, 3))
    feat2[..., 1::2] = np.transpose(out[64:], (1, 0, 2, 3))
    return feat2 + b_conv2[None, :, None, None]


# ---------------- RoIAlign grid (host) ----------------
def roi_grid_host(rois):
    nroi = rois.shape[0]
    W = H = 32
    x1 = rois[:, 1] * SCALE; y1 = rois[:, 2] * SCALE
    x2 = rois[:, 3] * SCALE; y2 = rois[:, 4] * SCALE
    bw = np.maximum(x2 - x1, 1.0) / P
    bh = np.maximum(y2 - y1, 1.0) / P
    grid = np.arange(P, dtype=np.float32) + 0.5
    sx = x1[:, None, None] + bw[:, None, None] * grid[None, None, :]
    sy = y1[:, None, None] + bh[:, None, None] * grid[None, :, None]
    sx = np.broadcast_to(sx, (nroi, P, P)).reshape(-1)
    sy = np.broadcast_to(sy, (nroi, P, P)).reshape(-1)
    x0f = np.clip(np.floor(sx), 0, W - 1)
    y0f = np.clip(np.floor(sy), 0, H - 1)
    lx = np.clip(sx - x0f, 0.0, 1.0)
    ly = np.clip(sy - y0f, 0.0, 1.0)
    x0 = x0f.astype(np.int32); y0 = y0f.astype(np.int32)
    hi = x0 >= 31
    x0 = np.where(hi, 30, x0); lx = np.where(hi, 1.0, lx).astype(np.float32)
    hiy = y0 >= 31
    y0 = np.where(hiy, 30, y0); ly = np.where(hiy, 1.0, ly).astype(np.float32)
    img = np.repeat(np.arange(nroi, dtype=np.int32) // N, 16)
    j2 = x0 >> 1
    par = (x0 & 1).astype(np.float32)
    idx0 = img * 512 + y0 * 16 + j2
    idx1 = idx0 + 16
    w4 = np.stack([(1 - ly) * (1 - lx), (1 - ly) * lx, ly * (1 - lx), ly * lx], 1).astype(np.float32)
    return np.stack([idx0, idx1], 1).astype(np.int32), w4, par


def roi_w6_host(rois):
    idx, w4, par = roi_grid_host(rois)
    wy0 = w4[:, 0] + w4[:, 1]
    wy1 = w4[:, 2] + w4[:, 3]
    lx = np.where(wy0 > 0, w4[:, 1] / np.maximum(wy0, 1e-30), w4[:, 3] / np.maximum(wy1, 1e-30))
    wa = (1 - par) * (1 - lx)
    wb = (1 - par) * lx + par * (1 - lx)
    wc = par * lx
    w6 = np.stack([wy0 * wa, wy0 * wb, wy0 * wc, wy1 * wa, wy1 * wb, wy1 * wc], 1)
    return idx, w6.astype(np.float32)


def feat2_rows_host(feat2r):
    rows = np.transpose(feat2r.reshape(-1, 64, 32, 16, 2), (0, 2, 3, 4, 1)).reshape(-1, 128)
    return np.concatenate([rows, np.zeros((2, 128), rows.dtype)], 0)


def roi_align_host(feat2r, rois):
    idx, w6 = roi_w6_host(rois)
    rows = feat2_rows_host(feat2r)
    g = rows.reshape(-1)
    npt = idx.shape[0]
    blk = np.empty((npt, 2, 256), np.float32)
    for r in range(2):
        for p in range(npt):
            st = idx[p, r] * 128
            blk[p, r] = g[st: st + 256]
    offs = [0, 64, 128, 256, 320, 384]
    b2 = blk.reshape(npt, 512)
    pooled = np.zeros((npt, 64), np.float32)
    for s in range(6):
        pooled += w6[:, s:s + 1] * b2[:, offs[s]: offs[s] + 64]
    return pooled


# ---------------- GNN (host mirror) ----------------
def mask_host(coor, r):
    bm = np.zeros((NROW, NROW), np.float32)
    for b in range(BC):
        d = np.linalg.norm(coor[b][:, None, :] - coor[b][None, :, :], axis=-1)
        m = (d <= (r[b][:, None] + r[b][None, :])) & ~np.eye(N, dtype=bool)
        bm[b * N:(b + 1) * N, b * N:(b + 1) * N] = m
    return bm, bm.sum(1)


def internet_host(s, bm, deg, p):
    sw, sb, rw, rb, aw, ab, ow, ob = p
    Wl, Wr = rw[:, :D], rw[:, D:]
    self_d = s @ sw.T + sb
    u = s @ Wl.T + rb
    v = s @ Wr.T
    rel = deg[:, None] * u + bm @ v
    a = np.maximum((self_d + rel) @ aw.T + ab, 0)
    return np.maximum(a @ ow[:, :D].T + s @ ow[:, D:].T + ob, 0)


def gnn_host(obj_t, src_coor, r, inputs):
    states = list(obj_t)
    masks = [mask_host(src_coor[:, t], r) for t in range(4)]
    num_rollouts = int(inputs['num_rollouts'])
    out = []
    for rr in range(num_rollouts):
        cs = []
        for k in range(4):
            p = (inputs['g_self_w'][k], inputs['g_self_b'][k], inputs['g_rel_w'][k],
                 inputs['g_rel_b'][k], inputs['g_aff_w'][k], inputs['g_aff_b'][k],
                 inputs['g_out_w'][k], inputs['g_out_b'][k])
            bm, deg = masks[k]
            cs.append(internet_host(states[k], bm, deg, p))
        s = np.concatenate(cs, -1) @ inputs['agg_w'].T + inputs['agg_b']
        bbox = s @ inputs['dec_w'].T + inputs['dec_b']
        out.append(bbox.reshape(BC, N, 4))
        states = states[1:] + [s]
        coor = bbox[:, 2:].reshape(BC, N, 2)
        masks = masks[1:] + [mask_host(coor, r)]
    return np.stack(out, 1)


def full_host(inputs, shard):
    sl = slice(shard * BC, (shard + 1) * BC)
    x = inputs['x'][sl].reshape(NIMG, CIN, IMG, IMG)
    rois = inputs['rois'][sl].reshape(NROI, 5)
    coor = inputs['src_coor_features'][sl]
    r = (((rois.reshape(BC, T, N, 5)[..., 4] - rois.reshape(BC, T, N, 5)[..., 2]) / 2
          + (rois.reshape(BC, T, N, 5)[..., 3] - rois.reshape(BC, T, N, 5)[..., 1]) / 2) / 2).mean(1)
    f1 = np.maximum(conv1_host(x, inputs['w_conv1'], inputs['b_conv1']), 0)
    f2 = np.maximum(conv2_host(f1, inputs['w_conv2'], inputs['b_conv2']), 0)
    pooled = roi_align_host(f2, rois)
    pool_cp = pooled.reshape(NROI, 16, 64)
    Wp = inputs['fc0_w'].reshape(D, 64, 16)
    obj = np.einsum('rpc,dcp->rd', pool_cp, Wp) + inputs['fc0_b']
    obj = np.maximum(obj, 0)
    emb = np.maximum(coor.reshape(NROI, 2) @ inputs['fc0c_w'].T + inputs['fc0c_b'], 0)
    emb = np.maximum(emb @ inputs['fc1c_w'].T + inputs['fc1c_b'], 0)
    o2 = np.maximum(obj @ inputs['red_w'][:, :D].T + emb @ inputs['red_w'][:, D:].T
                    + inputs['red_b'], 0)
    o2 = o2.reshape(BC, T, N, D)
    obj_t = [o2[:, t].reshape(NROW, D) for t in range(4)]
    return gnn_host(obj_t, coor, r, inputs)


# ---------------- device input packing ----------------
def make_core_inputs(inputs, shard):
    import ml_dtypes
    bf16 = ml_dtypes.bfloat16
    sl = slice(shard * BC, (shard + 1) * BC)
    x = np.asarray(inputs['x'][sl], np.float32).reshape(NIMG, CIN, IMG, IMG)
    rois = np.asarray(inputs['rois'][sl], np.float32).reshape(NROI, 5)
    coor = np.asarray(inputs['src_coor_features'][sl], np.float32)   # [BC,T,N,2]
    rr5 = rois.reshape(BC, T, N, 5)
    r = (((rr5[..., 4] - rr5[..., 2]) / 2 + (rr5[..., 3] - rr5[..., 1]) / 2) / 2).mean(1)

    d = {}
    cols = conv1_im2col_host(x)                       # [45, NIMG, 64, 32]
    cols2 = cols.reshape(45, NIMG, 2048)
    # split each image's 2048 output cols in half; halves at partition
    # blocks 0:45 and 64:109 drive two concurrent PE row-groups
    d['im2col45'] = np.concatenate(
        [cols2[:, :, :1024], cols2[:, :, 1024:]], 0).reshape(90, -1).astype(bf16)
    w1h = conv1_weights_host(np.asarray(inputs['w_conv1']))
    w1p = np.zeros((128, 128), np.float32)
    w1p[0:45] = w1h
    w1p[64:109] = w1h
    d['w1'] = w1p.astype(bf16)
    b1 = np.asarray(inputs['b_conv1'], np.float32)
    d['b1'] = np.tile(b1, 2).reshape(128, 1).astype(np.float32)
    Wb = conv2_weights_host(np.asarray(inputs['w_conv2']))
    d['w2a'] = np.stack([Wb[dy][0] for dy in range(3)]).astype(bf16)
    d['w2b'] = np.stack([Wb[dy][1] for dy in range(3)]).astype(bf16)
    d['w2c'] = np.stack([Wb[dy][2] for dy in range(3)]).astype(bf16)
    b2 = np.asarray(inputs['b_conv2'], np.float32)
    d['b2'] = np.tile(b2, 2).reshape(128, 1).astype(np.float32)

    idx, w6 = roi_w6_host(rois)                       # [NPT,2] int32, [NPT,6]
    d['gidx'] = idx.reshape(NG, 128, 2).transpose(1, 0, 2).reshape(128, NG * 2).copy()
    d['w6'] = w6.reshape(NG, 128, 6).transpose(1, 0, 2).reshape(128, NG * 6).astype(np.float32)

    fc0w = np.asarray(inputs['fc0_w'], np.float32).reshape(D, 64, 16)
    d['fc0t'] = np.ascontiguousarray(fc0w.transpose(2, 1, 0)).astype(bf16)  # [pt, c, d]
    d['fc0b'] = np.asarray(inputs['fc0_b'], np.float32).reshape(2, 128).T.copy()

    d['coor_fm'] = coor.reshape(NROI, 2).T.astype(bf16).copy()

    def t2(w):   # [256, K] -> [kc, 128, 256] lhsT chunks (w.T row-chunks)
        wT = np.ascontiguousarray(np.asarray(w, np.float32).T)       # [K, 256]
        K = wT.shape[0]
        return wT.reshape(K // 128, 128, 256).astype(bf16)

    def bcol(b):  # [256] -> [128, 2]
        return np.asarray(b, np.float32).reshape(2, 128).T.copy()

    d['fc0ct'] = np.asarray(inputs['fc0c_w'], np.float32).T.astype(bf16).copy()  # [2, 256]
    d['fc0cb'] = bcol(inputs['fc0c_b'])
    d['fc1ct'] = t2(inputs['fc1c_w'])
    d['fc1cb'] = bcol(inputs['fc1c_b'])
    redw = np.asarray(inputs['red_w'], np.float32)
    d['redoT'] = t2(redw[:, :D])
    d['redeT'] = t2(redw[:, D:])
    d['redb'] = bcol(inputs['red_b'])

    # aff layer fused into stage 1: aw@ commutes with deg-scaling and bm-mixing
    aw = np.asarray(inputs['g_aff_w'], np.float32)     # [4, 256, 256]
    d['gswT'] = np.stack([t2(aw[k] @ np.asarray(inputs['g_self_w'][k], np.float32))
                          for k in range(4)])
    gow = np.asarray(inputs['g_out_w'], np.float32)
    d['gowaT'] = np.stack([t2(gow[k][:, :D]) for k in range(4)])
    d['gowsT'] = np.stack([t2(gow[k][:, D:]) for k in range(4)])
    d['aggT'] = t2(inputs['agg_w'])                    # [8, 128, 256]
    decw = np.asarray(inputs['dec_w'], np.float32)     # [4, 256]
    d['decT'] = decw.T.reshape(2, 128, 4).astype(bf16).copy()

    # g1T: per (k, kc): [(aw@Wl)T_kc | (aw@Wr)T_kc] -> [128, 4k*2kc*512]
    grw = np.asarray(inputs['g_rel_w'], np.float32)    # [4, 256, 512]
    blocks = []
    for k in range(4):
        Wl = (aw[k] @ grw[k][:, :D]).T                 # [256 d, 256 m]
        Wr = (aw[k] @ grw[k][:, D:]).T
        for kc in range(2):
            blocks.append(np.concatenate(
                [Wl[kc * 128:(kc + 1) * 128], Wr[kc * 128:(kc + 1) * 128]], axis=1))
    d['g1T'] = np.concatenate(blocks, axis=1).astype(bf16)     # [128, 4096]

    # bias rows for rank-1/2 bias matmuls (aff bias composed in)
    d['rbrow'] = np.concatenate(
        [(aw[k] @ np.asarray(inputs['g_rel_b'][k], np.float32)).reshape(1, 256)
         for k in range(4)], axis=1).astype(bf16)               # [1, 1024]
    d['sb2'] = np.concatenate(
        [(aw[k] @ np.asarray(inputs['g_self_b'][k], np.float32)
          + np.asarray(inputs['g_aff_b'][k], np.float32)).reshape(2, 128)
         for k in range(4)], axis=1).astype(bf16)               # [2, 512]
    d['ob2'] = np.concatenate([np.asarray(inputs['g_out_b'][k], np.float32).reshape(2, 128)
                               for k in range(4)], axis=1).astype(bf16)
    d['aggb2'] = np.asarray(inputs['agg_b'], np.float32).reshape(2, 128).astype(bf16)
    d['decbrow'] = np.asarray(inputs['dec_b'], np.float32).reshape(1, 4).astype(bf16)
    blk2 = np.zeros((2, 96), np.float32)
    blk2[0, :48] = 1.0; blk2[1, 48:] = 1.0
    d['blk2'] = blk2.astype(bf16)

    # masks: bm at rows 0:48, diag(deg) at rows 64:112 (partition-base rules)
    hm2 = []
    for m in range(4):
        bm, deg = mask_host(coor[:, m], r)
        hm = np.zeros((128, 48), np.float32)
        hm[0:48] = bm
        hm[64:112] = np.diag(deg)
        hm2.append(hm.astype(bf16))
    d['hm2'] = np.stack(hm2)                           # [4, 128, 48]
    Tmat = np.full((NROW, NROW), -1.0, np.float32)
    for b in range(BC):
        rs = (r[b][:, None] + r[b][None, :]) ** 2
        np.fill_diagonal(rs, -1.0)
        Tmat[b * N:(b + 1) * N, b * N:(b + 1) * N] = rs
    d['Tm'] = Tmat
    d['ones48'] = np.ones((48, 128), bf16)
    d['ones2'] = np.ones((2, 48), bf16)
    d['ident'] = np.eye(128, dtype=bf16)
    return d


dt = mybir.dt
AF = mybir.ActivationFunctionType
OP = mybir.AluOpType

NIMG, NROI, NROW, NPT = 32, 192, 48, 3072
NG = 24            # gather groups (128 pts each)
IMG_GRP = 4        # images per conv group
NGRP = NIMG // IMG_GRP
IMGF = 2 * 33 * 34  # free els per img: (py 2, Y 33, Xp 2, Xh 17) X-parity split

# gather group gg may be issued after conv2 of image (8*gg+7)//6 + 1
# (+1 because the y0+1 row window of the last position can touch the
#  first row of the next image; weight there is 0 but it must be written)
GATHER_AFTER_IMG = {}
for _gg in range(NG):
    GATHER_AFTER_IMG.setdefault(min((8 * _gg + 7) // 6 + 1, NIMG - 1), []).append(_gg)


def build(nc: bass.Bass, zero_bias=False):
    f32, bf16, i32 = dt.float32, dt.bfloat16, dt.int32

    def din(name, shape, d):
        return nc.dram_tensor(name, shape, d, kind="ExternalInput")

    im2col = din("im2col45", [90, 32768], bf16)
    w1 = din("w1", [128, 128], bf16)
    b1 = din("b1", [128, 1], f32)
    w2a = din("w2a", [3, 128, 128], bf16)
    w2b = din("w2b", [3, 128, 128], bf16)
    w2c = din("w2c", [3, 64, 128], bf16)
    b2 = din("b2", [128, 1], f32)
    gidx = din("gidx", [128, 48], i32)
    w6 = din("w6", [128, 144], f32)
    fc0t = din("fc0t", [16, 64, 256], bf16)
    fc0b = din("fc0b", [128, 2], f32)
    coor = din("coor_fm", [2, 192], bf16)
    fc0ct = din("fc0ct", [2, 256], bf16)
    fc0cb = din("fc0cb", [128, 2], f32)
    fc1ct = din("fc1ct", [2, 128, 256], bf16)
    fc1cb = din("fc1cb", [128, 2], f32)
    redoT = din("redoT", [2, 128, 256], bf16)
    redeT = din("redeT", [2, 128, 256], bf16)
    redb = din("redb", [128, 2], f32)
    gswT = din("gswT", [4, 2, 128, 256], bf16)
    gowaT = din("gowaT", [4, 2, 128, 256], bf16)
    gowsT = din("gowsT", [4, 2, 128, 256], bf16)
    g1T = din("g1T", [128, 4096], bf16)
    rbrow = din("rbrow", [1, 1024], bf16)
    sb2 = din("sb2", [2, 512], bf16)
    ob2 = din("ob2", [2, 512], bf16)
    aggb2 = din("aggb2", [2, 128], bf16)
    decbrow = din("decbrow", [1, 4], bf16)
    blk2 = din("blk2", [2, 96], bf16)
    aggT = din("aggT", [8, 128, 256], bf16)
    decT = din("decT", [2, 128, 4], bf16)
    hm2 = din("hm2", [4, 128, 48], bf16)
    Tm = din("Tm", [48, 48], f32)
    ones48 = din("ones48", [48, 128], bf16)
    ones2 = din("ones2", [2, 48], bf16)
    ident = din("ident", [128, 128], bf16)

    out = nc.dram_tensor("bbox_out", [8, 8, 6, 4], f32, kind="ExternalOutput")

    with tile.TileContext(nc) as tc, ExitStack() as ctx:
        # ---- persistent pools ----
        wp = ctx.enter_context(tc.tile_pool(name="w", bufs=1))
        dramp = ctx.enter_context(tc.tile_pool(name="dram", bufs=1, space="DRAM"))
        sp = ctx.enter_context(tc.tile_pool(name="state", bufs=1))

        # conv-critical weights on the SP ring (small, before im2col loads)
        def loads(dram_t, shape, dtype, src_ap=None, eng=None):
            t = wp.tile(shape, dtype, tag=dram_t.name)
            e = eng if eng is not None else nc.sync
            if src_ap is None:
                e.dma_start(t[:], dram_t[:, :])
            else:
                dims = [c for _, c in src_ap.ap[1:]]
                spec = " ".join(f"d{i}" for i in range(len(dims)))
                kw = {f"d{i}": dims[i] for i in range(len(dims) - 1)}
                dv = t[:].rearrange(f"p ({spec}) -> p {spec}", **kw)
                e.dma_start(dv, src_ap)
            return t

        w1_s = loads(w1, [128, 128], bf16)
        b1_s = loads(b1, [128, 1], f32)
        w2a_s = loads(w2a, [128, 3 * 128], bf16, w2a[:].rearrange("d p m -> p d m"))
        w2b_s = loads(w2b, [128, 3 * 128], bf16, w2b[:].rearrange("d p m -> p d m"))
        w2c_s = loads(w2c, [64, 3 * 128], bf16, w2c[:].rearrange("d p m -> p d m"))
        b2_s = loads(b2, [128, 1], f32)
        ident_s = loads(ident, [128, 128], bf16)

        # everything else on the ACT HWDGE ring (overlaps im2col loads)
        def load(dram_t, shape, dtype, src_ap=None):
            return loads(dram_t, shape, dtype, src_ap, eng=nc.scalar)

        gidx_s = load(gidx, [128, 48], i32)
        w6_s = load(w6, [128, 144], f32)
        fc0t_s = None  # loaded on the Pool ring inside the conv section
        fc0b_s = load(fc0b, [128, 2], f32)
        coor_s = load(coor, [2, 192], bf16)
        fc0ct_s = load(fc0ct, [2, 256], bf16)
        fc0cb_s = load(fc0cb, [128, 2], f32)
        fc1ct_s = load(fc1ct, [128, 512], bf16, fc1ct[:].rearrange("k p m -> p k m"))
        fc1cb_s = load(fc1cb, [128, 2], f32)
        redoT_s = load(redoT, [128, 512], bf16, redoT[:].rearrange("k p m -> p k m"))
        redeT_s = load(redeT, [128, 512], bf16, redeT[:].rearrange("k p m -> p k m"))
        redb_s = load(redb, [128, 2], f32)

        # big GNN/fc weights go on the gpsimd (SWDGE) ring, emitted inside the
        # conv section AFTER the memsets (Pool is otherwise idle early)
        def loadp(dram_t, shape, dtype, src_ap=None):
            return loads(dram_t, shape, dtype, src_ap, eng=nc.gpsimd)

        def loadg(t):  # [4,2,128,256] -> [128, 4*512]
            return loadp(t, [128, 2048], bf16, t[:].rearrange("h k p m -> p h k m"))
        rbrow_s = load(rbrow, [1, 1024], bf16)
        sb2_s = load(sb2, [2, 512], bf16)
        ob2_s = load(ob2, [2, 512], bf16)
        aggb2_s = load(aggb2, [2, 128], bf16)
        decbrow_s = load(decbrow, [1, 4], bf16)
        blk2_s = load(blk2, [2, 96], bf16)
        aggT_s = None  # loaded on the Pool ring inside the conv section
        decT_s = load(decT, [128, 8], bf16, decT[:].rearrange("k p m -> p k m"))
        Tm_s = load(Tm, [48, 48], f32)
        ones48_s = load(ones48, [48, 128], bf16)
        ones2_s = load(ones2, [2, 48], bf16)

        # mask2 slots: bm rows 0:48, diag(deg) rows 64:112 (0..3 from host)
        mask2_t = [sp.tile([128, 48], bf16, name=f"m2_{m}", tag=f"m2_{m}") for m in range(11)]
        vu_t = [sp.tile([128, 256], bf16, name=f"vu{k}", tag=f"vu{k}") for k in range(4)]
        for m in range(11):
            nc.gpsimd.memset(mask2_t[m][32:64, :], 0.0)
        for k in range(4):
            nc.gpsimd.memset(vu_t[k][32:64, :], 0.0)
        for m in range(4):
            nc.scalar.dma_start(mask2_t[m][:], hm2[m])

        st = [sp.tile([128, 96], bf16, name=f"st{m}", tag=f"st{m}") for m in range(12)]
        bbox_sb = sp.tile([4, 384], f32, tag="bbox")
        poolT = sp.tile([64, 3072], bf16, tag="poolT")
        pba = sp.tile([128, NG * 64], bf16, tag="pba")
        fd = dramp.tile([16386, 128], bf16, tag="feat2")
        zpad = sp.tile([2, 128], bf16, tag="zpad")
        nc.gpsimd.memset(zpad[:], 0.0)
        nc.gpsimd.dma_start(fd[16384:16386, :], zpad[:])

        # ================= conv stage (+ interleaved roi gather) =============
        with ExitStack() as cvx:
            imcp = cvx.enter_context(tc.tile_pool(name="imc", bufs=2))
            f1p = cvx.enter_context(tc.tile_pool(name="f1", bufs=2))
            c1ps = cvx.enter_context(tc.tile_pool(name="c1ps", bufs=2, space="PSUM"))
            c2ps = cvx.enter_context(tc.tile_pool(name="c2ps", bufs=2, space="PSUM"))
            tps = cvx.enter_context(tc.tile_pool(name="tps", bufs=1, space="PSUM"))
            f2p = cvx.enter_context(tc.tile_pool(name="f2", bufs=3))
            gp = cvx.enter_context(tc.tile_pool(name="g", bufs=3))
            bp = cvx.enter_context(tc.tile_pool(name="bil", bufs=3))

            def roi_gather_group(g):
                gb = gp.tile([128, 512], bf16, tag="gb")
                for rrow in range(2):
                    nc.gpsimd.indirect_dma_start(
                        out=gb[:, rrow * 256:(rrow + 1) * 256], out_offset=None, in_=fd[:],
                        in_offset=bass.IndirectOffsetOnAxis(
                            ap=gidx_s[:, 2 * g + rrow:2 * g + rrow + 1], axis=0))
                offs = [0, 64, 128, 256, 320, 384]
                a0 = bp.tile([128, 64], f32, tag="acc0")
                a1 = bp.tile([128, 64], f32, tag="acc1")
                nc.vector.tensor_scalar(out=a0[:], in0=gb[:, 0:64],
                                        scalar1=w6_s[:, 6 * g:6 * g + 1], scalar2=None, op0=OP.mult)
                cur, nxt = a0, a1
                for s in range(1, 6):
                    dst = pba[:, 64 * g:64 * (g + 1)] if s == 5 else nxt[:]
                    nc.vector.scalar_tensor_tensor(
                        out=dst, in0=gb[:, offs[s]:offs[s] + 64],
                        scalar=w6_s[:, 6 * g + s:6 * g + s + 1], in1=cur[:],
                        op0=OP.mult, op1=OP.add)
                    if s < 5:
                        cur, nxt = nxt, cur

            f1_t = [sp.tile([128, IMG_GRP * IMGF], bf16, name=f"f1_{j}", tag=f"f1_{j}")
                    for j in range(2)]
            for j in range(2):
                fv = f1_t[j][:].rearrange("p (i py y xp xh) -> p i py y xp xh",
                                          i=IMG_GRP, py=2, y=33, xp=2)
                nc.gpsimd.memset(fv[:, :, :, 32:33, :, :], 0.0)   # Y halo row
                nc.gpsimd.memset(fv[:, :, :, :, 0:1, 16:17], 0.0)  # X halo col (Xp0)
            gswT_s = loadg(gswT)
            gowaT_s, gowsT_s = loadg(gowaT), loadg(gowsT)
            g1T_s = loadp(g1T, [128, 4096], bf16)
            aggT_s = loadp(aggT, [128, 2048], bf16,
                           aggT[:].rearrange("k p m -> p k m"))
            fc0t_s = loadp(fc0t, [64, 16 * 256], bf16,
                           fc0t[:].rearrange("t p m -> p t m"))
            for g in range(NGRP):
                imc = imcp.tile([128, IMG_GRP * 1024], bf16, tag="imc")
                csl = slice(g * IMG_GRP * 1024, (g + 1) * IMG_GRP * 1024)
                nc.sync.dma_start(imc[0:45, :], im2col[0:45, csl])
                nc.sync.dma_start(imc[64:109, :], im2col[45:90, csl])
                f1 = f1_t[g % 2]
                f1h = f1[:].rearrange("p (i py y xp xh) -> p i py y xp xh",
                                      i=IMG_GRP, py=2, y=33, xp=2)
                for i in range(IMG_GRP):
                    pv = [c1ps.tile([128, 1024], f32, name=f"c1_{h}", tag="c1")
                          for h in range(2)]
                    for q in range(2):
                        for h in range(2):
                            base = 0 if h == 0 else 64
                            nc.tensor.matmul(
                                pv[h][:, q * 512:(q + 1) * 512],
                                lhsT=w1_s[base:base + 45, :],
                                rhs=imc[base:base + 45,
                                        i * 1024 + q * 512:i * 1024 + (q + 1) * 512],
                                start=True, stop=True)
                    for h in range(2):
                        psv = pv[h][:].rearrange("p (y j) -> p y j", y=32)
                        for py in range(2):
                            dst = f1h[:, i, py, 16 * h:16 * h + 16, :, 0:16]
                            src = psv[:, py::2, :].rearrange(
                                "p y (xh xp) -> p y xp xh", xp=2)
                            if py:
                                nc.vector.tensor_scalar(
                                    out=dst, in0=src, scalar1=b1_s[:, 0:1],
                                    scalar2=0.0, op0=OP.add, op1=OP.max)
                            else:
                                nc.scalar.activation(out=dst, in_=src,
                                                     func=AF.Relu, bias=b1_s[:, 0:1])
                for i in range(IMG_GRP):
                    ps = c2ps.tile([128, 512], f32, tag="c2")
                    first = True
                    for dy in range(3):
                        py, yo = dy % 2, dy // 2
                        for gsel in range(3):
                            xp, xh = gsel % 2, gsel // 2
                            rhs_ap = f1h[:, i, py, yo:yo + 32, xp, xh:xh + 16]
                            if gsel == 2:
                                rhs_ap = rhs_ap[0:64]
                                lhsT = w2c_s[:, dy * 128:(dy + 1) * 128]
                            else:
                                lhsT = (w2a_s if gsel == 0 else w2b_s)[:, dy * 128:(dy + 1) * 128]
                            nc.tensor.matmul(ps[:], lhsT=lhsT, rhs=rhs_ap,
                                             start=first, stop=(dy == 2 and gsel == 2))
                            first = False
                    f2s = f2p.tile([128, 512], bf16, tag="f2s")
                    if i % 2 == 0:
                        nc.vector.tensor_scalar(out=f2s[:], in0=ps[:], scalar1=b2_s[:, 0:1],
                                                scalar2=0.0, op0=OP.add, op1=OP.max)
                    else:
                        nc.scalar.activation(out=f2s[:], in_=ps[:], func=AF.Relu,
                                             bias=b2_s[:, 0:1])
                    tp = tps.tile([128, 512], bf16, tag="tp")
                    for b in range(4):
                        nc.tensor.transpose(tp[:, b * 128:(b + 1) * 128],
                                            f2s[:, b * 128:(b + 1) * 128], ident_s[:])
                    f2t = f2p.tile([128, 512], bf16, tag="f2t")
                    if i % 2 == 0:
                        nc.scalar.activation(out=f2t[:], in_=tp[:], func=AF.Copy)
                    else:
                        nc.vector.tensor_copy(out=f2t[:], in_=tp[:])
                    img = g * IMG_GRP + i
                    dst = fd[img * 512:(img + 1) * 512, :].rearrange(
                        "(b p) c -> p b c", p=128)
                    nc.sync.dma_start(dst, f2t[:].rearrange("p (b c) -> p b c", c=128))
                    for gg in GATHER_AFTER_IMG.get(img, []):
                        roi_gather_group(gg)

        # ================= pool transpose + fc0 + emb + red =================
        with ExitStack() as gx:
            ops = gx.enter_context(tc.tile_pool(name="ops", bufs=2, space="PSUM"))
            ptp2 = gx.enter_context(tc.tile_pool(name="ptp2", bufs=2, space="PSUM"))

            # transposes decoupled from the conv loop: they would otherwise
            # stall the in-order PE queue on the gather->bilinear chains
            for g in range(NG):
                pt = ptp2.tile([64, 128], bf16, tag="pt")
                nc.tensor.transpose(pt[:], pba[:, 64 * g:64 * (g + 1)], ident_s[:])
                if g % 2:
                    nc.vector.tensor_copy(out=poolT[:, 128 * g:128 * (g + 1)], in_=pt[:])
                else:
                    nc.scalar.activation(out=poolT[:, 128 * g:128 * (g + 1)],
                                         in_=pt[:], func=AF.Copy)

            obj = sp.tile([128, 384], bf16, tag="obj")
            pview = poolT[:].rearrange("p (r t) -> p t r", t=16)
            for m2 in range(2):
                ps = ops.tile([128, 192], f32, tag="obj")
                for pt_i in range(16):
                    nc.tensor.matmul(ps[:], lhsT=fc0t_s[:, pt_i * 256 + m2 * 128:
                                                        pt_i * 256 + m2 * 128 + 128],
                                     rhs=pview[:, pt_i, :],
                                     start=(pt_i == 0), stop=(pt_i == 15))
                nc.scalar.activation(out=obj[:, m2 * 192:(m2 + 1) * 192], in_=ps[:],
                                     func=AF.Relu, bias=fc0b_s[:, m2:m2 + 1])
            emb1 = sp.tile([128, 384], bf16, tag="emb1")
            for m2 in range(2):
                ps = ops.tile([128, 192], f32, tag="emb")
                nc.tensor.matmul(ps[:], lhsT=fc0ct_s[:, m2 * 128:(m2 + 1) * 128],
                                 rhs=coor_s[:], start=True, stop=True)
                nc.scalar.activation(out=emb1[:, m2 * 192:(m2 + 1) * 192], in_=ps[:],
                                     func=AF.Relu, bias=fc0cb_s[:, m2:m2 + 1])
            emb2 = sp.tile([128, 384], bf16, tag="emb2")
            for m2 in range(2):
                ps = ops.tile([128, 192], f32, tag="emb")
                for kc in range(2):
                    nc.tensor.matmul(ps[:], lhsT=fc1ct_s[:, kc * 256 + m2 * 128:
                                                         kc * 256 + m2 * 128 + 128],
                                     rhs=emb1[:, kc * 192:(kc + 1) * 192],
                                     start=(kc == 0), stop=(kc == 1))
                nc.scalar.activation(out=emb2[:, m2 * 192:(m2 + 1) * 192], in_=ps[:],
                                     func=AF.Relu, bias=fc1cb_s[:, m2:m2 + 1])
            o2 = sp.tile([128, 384], bf16, tag="o2")
            for m2 in range(2):
                ps = ops.tile([128, 192], f32, tag="o2")
                for kc in range(2):
                    nc.tensor.matmul(ps[:], lhsT=redoT_s[:, kc * 256 + m2 * 128:
                                                         kc * 256 + m2 * 128 + 128],
                                     rhs=obj[:, kc * 192:(kc + 1) * 192],
                                     start=(kc == 0), stop=False)
                for kc in range(2):
                    nc.tensor.matmul(ps[:], lhsT=redeT_s[:, kc * 256 + m2 * 128:
                                                         kc * 256 + m2 * 128 + 128],
                                     rhs=emb2[:, kc * 192:(kc + 1) * 192],
                                     start=False, stop=(kc == 1))
                nc.scalar.activation(out=o2[:, m2 * 192:(m2 + 1) * 192], in_=ps[:],
                                     func=AF.Relu, bias=redb_s[:, m2:m2 + 1])
            o2v = o2[:].rearrange("p (m2 b t n) -> p m2 b t n", m2=2, b=8, t=4)
            for m in range(4):
                nc.vector.tensor_copy(
                    out=st[m][:].rearrange("p (m2 b n) -> p m2 b n", m2=2, b=8),
                    in_=o2v[:, :, :, m, :])

        # ================= GNN rollouts (v2) =================
        with ExitStack() as rx:
            p1p = rx.enter_context(tc.tile_pool(name="p1p", bufs=3, space="PSUM"))
            pxp = rx.enter_context(tc.tile_pool(name="pxp", bufs=1, space="PSUM"))
            msp = rx.enter_context(tc.tile_pool(name="msp", bufs=1, space="PSUM"))
            hb = rx.enter_context(tc.tile_pool(name="hbuf", bufs=4))
            cb = rx.enter_context(tc.tile_pool(name="cbuf", bufs=5))

            onesrow = ones2_s[0:1, :]

            # dec (bbox) + mask build for rollout prr, emitted DEFERRED inside
            # rollout prr+1 so the DVE round-trips hide behind independent
            # pair matmuls instead of stalling the in-order PE queue between
            # rollouts.
            def emit_dec(prr):
                s_new = st[prr + 4]
                ms = msp.tile([48, 144], f32, tag="ms")
                d_ps = ms[0:4, 0:48]
                for kc in range(2):
                    nc.tensor.matmul(d_ps[:], lhsT=decT_s[:, kc * 4:kc * 4 + 4],
                                     rhs=s_new[:, kc * 48:kc * 48 + 48],
                                     start=(kc == 0), stop=zero_bias and kc == 1)
                if not zero_bias:
                    nc.tensor.matmul(d_ps[:], lhsT=decbrow_s[:], rhs=onesrow,
                                     start=False, stop=True)
                bbv = bbox_sb[:].rearrange("f (b q) -> f b q", b=8)[:, :, prr * 6:prr * 6 + 6]
                nc.vector.tensor_copy(out=bbv, in_=d_ps[:])
                if prr >= 7:
                    return None
                d2_ps = ms[0:2, 48:96]
                for kc in range(2):
                    nc.tensor.matmul(d2_ps[:], lhsT=decT_s[:, kc * 4 + 2:kc * 4 + 4],
                                     rhs=s_new[:, kc * 48:kc * 48 + 48],
                                     start=(kc == 0), stop=zero_bias and kc == 1)
                if not zero_bias:
                    nc.tensor.matmul(d2_ps[:], lhsT=decbrow_s[:, 2:4], rhs=onesrow,
                                     start=False, stop=True)
                coorb = hb.tile([2, 48], bf16, tag="coorb")
                nc.vector.tensor_copy(out=coorb[:], in_=d2_ps[:])
                cm2 = hb.tile([2, 48], bf16, tag="cm2")
                nc.vector.tensor_scalar(out=cm2[:], in0=coorb[:], scalar1=-2.0,
                                        scalar2=None, op0=OP.mult)
                sq = hb.tile([2, 48], bf16, tag="sq")
                nc.vector.tensor_tensor(out=sq[:], in0=coorb[:], in1=coorb[:], op=OP.mult)
                return ms, coorb, cm2, sq

            def emit_mask(prr, decst):
                m = prr + 4
                ms, coorb, cm2, sq = decst
                m_ps = ms[0:48, 96:144]
                nc.tensor.matmul(m_ps[:], lhsT=coorb[:], rhs=cm2[:], start=True, stop=False)
                nc.tensor.matmul(m_ps[:], lhsT=sq[:], rhs=ones2_s[:], start=False, stop=False)
                nc.tensor.matmul(m_ps[:], lhsT=ones2_s[:], rhs=sq[:], start=False, stop=True)
                nc.vector.tensor_tensor(out=mask2_t[m][0:48, :], in0=m_ps[:], in1=Tm_s[:],
                                        op=OP.is_le)
                degc = hb.tile([48, 1], f32, tag="degc")
                nc.vector.tensor_reduce(out=degc[:], in_=mask2_t[m][0:48, :],
                                        axis=mybir.AxisListType.X, op=OP.add)
                nc.vector.tensor_scalar(out=mask2_t[m][64:112, :],
                                        in0=ident_s[0:48, 0:48],
                                        scalar1=degc[:, 0:1], scalar2=None,
                                        op0=OP.mult)

            for rr in range(8):
                ps1 = {}
                vu = {}
                xs = {}
                asb = {}
                cs = {}
                # --- step1: u' | v  (row-major, stationary = state chunks) ---
                for k in range(4):
                    m = rr + k
                    s = st[m]
                    p1 = p1p.tile([48, 512], f32, tag="p1")
                    nc.tensor.matmul(p1[:], lhsT=s[:, 0:48],
                                     rhs=g1T_s[:, k * 1024:k * 1024 + 512],
                                     start=True, stop=False)
                    if not zero_bias:
                        nc.tensor.matmul(p1[:, 0:256], lhsT=onesrow,
                                         rhs=rbrow_s[:, k * 256:(k + 1) * 256],
                                         start=False, stop=False)
                    nc.tensor.matmul(p1[:], lhsT=s[:, 48:96],
                                     rhs=g1T_s[:, k * 1024 + 512:(k + 1) * 1024],
                                     start=False, stop=True)
                    ps1[k] = p1
                # --- evac1: vu = v rows 0:48, u' rows 64:112, bf16 ---
                for k in range(4):
                    t = vu_t[k]
                    nc.vector.tensor_copy(out=t[0:48, :], in_=ps1[k][:, 256:512])
                    nc.scalar.activation(out=t[64:112, :], in_=ps1[k][:, 0:256], func=AF.Copy)
                    vu[k] = t
                # --- deferred dec of the previous rollout ---
                if rr > 0:
                    decst = emit_dec(rr - 1)
                # --- psum_x: self + mask2-injection + bias ---
                pxt = pxp.tile([128, 384], f32, tag="px")
                for k in range(4):
                    if k == 3 and rr > 0:
                        # mask for slot rr+3 right before its first use; its
                        # DVE inputs were computed during the k=0..2 matmuls
                        emit_mask(rr - 1, decst)
                    m = rr + k
                    s = st[m]
                    px = pxt[:, k * 96:(k + 1) * 96]
                    for m2 in range(2):
                        for kc in range(2):
                            lo = k * 512 + kc * 256 + m2 * 128
                            nc.tensor.matmul(px[:, m2 * 48:m2 * 48 + 48],
                                             lhsT=gswT_s[:, lo:lo + 128],
                                             rhs=s[:, kc * 48:kc * 48 + 48],
                                             start=(k == 0 and m2 == 0 and kc == 0),
                                             stop=False)
                    for m2 in range(2):
                        nc.tensor.matmul(px[:, m2 * 48:m2 * 48 + 48],
                                         lhsT=vu[k][0:112, m2 * 128:(m2 + 1) * 128],
                                         rhs=mask2_t[m][0:112, :],
                                         start=False,
                                         stop=(zero_bias and k == 3 and m2 == 1))
                    if not zero_bias:
                        nc.tensor.matmul(px[:], lhsT=sb2_s[:, k * 128:(k + 1) * 128],
                                         rhs=blk2_s[:], start=False, stop=(k == 3))
                    xs[k] = px
                # --- a = relu(a_pre): aff was composed into stage-1 weights ---
                for k in range(4):
                    t = hb.tile([128, 96], bf16, tag="asb")
                    if k % 2:
                        nc.scalar.activation(out=t[:], in_=xs[k][:], func=AF.Relu)
                    else:
                        nc.vector.tensor_scalar(out=t[:], in0=xs[k][:], scalar1=0.0,
                                                scalar2=None, op0=OP.max)
                    asb[k] = t
                # --- out ---
                pot = pxp.tile([128, 384], f32, tag="po")
                for k in range(4):
                    m = rr + k
                    s = st[m]
                    po = pot[:, k * 96:(k + 1) * 96]
                    for m2 in range(2):
                        for kc in range(2):
                            lo = k * 512 + kc * 256 + m2 * 128
                            nc.tensor.matmul(po[:, m2 * 48:m2 * 48 + 48],
                                             lhsT=gowaT_s[:, lo:lo + 128],
                                             rhs=asb[k][:, kc * 48:kc * 48 + 48],
                                             start=(k == 0 and m2 == 0 and kc == 0),
                                             stop=False)
                            nc.tensor.matmul(po[:, m2 * 48:m2 * 48 + 48],
                                             lhsT=gowsT_s[:, lo:lo + 128],
                                             rhs=s[:, kc * 48:kc * 48 + 48],
                                             start=False,
                                             stop=(zero_bias and k == 3
                                                   and kc == 1 and m2 == 1))
                    if not zero_bias:
                        nc.tensor.matmul(po[:], lhsT=ob2_s[:, k * 128:(k + 1) * 128],
                                         rhs=blk2_s[:], start=False, stop=(k == 3))
                    cs[k] = po
                for k in range(4):
                    t = cb.tile([128, 96], bf16, tag=f"csb{k}")
                    nc.scalar.activation(out=t[:], in_=cs[k][:], func=AF.Relu)
                    cs[k] = t
                # --- agg ---
                g_ps = pxp.tile([128, 96], f32, tag="g")
                for m2 in range(2):
                    n = 0
                    for k in range(4):
                        for kc in range(2):
                            lo = (k * 2 + kc) * 256 + m2 * 128
                            nc.tensor.matmul(g_ps[:, m2 * 48:m2 * 48 + 48],
                                             lhsT=aggT_s[:, lo:lo + 128],
                                             rhs=cs[k][:, kc * 48:kc * 48 + 48],
                                             start=(m2 == 0 and n == 0),
                                             stop=(zero_bias and m2 == 1 and n == 7))
                            n += 1
                if not zero_bias:
                    nc.tensor.matmul(g_ps[:], lhsT=aggb2_s[:], rhs=blk2_s[:],
                                     start=False, stop=True)
                s_new = st[rr + 4]
                nc.vector.tensor_copy(out=s_new[:], in_=g_ps[:])
            emit_dec(7)
            nc.sync.dma_start(
                out[:].rearrange("b rr n f -> f (b rr n)"), bbox_sb[:])
    return nc


_NC = None
_NC_ZB = None


def _get_nc(zero_bias=False):
    global _NC, _NC_ZB
    if _NC is None or _NC_ZB != zero_bias:
        nc = bass.Bass()
        build(nc, zero_bias=zero_bias)
        split_drain_waits(nc)
        _NC = nc
        _NC_ZB = zero_bias
    return _NC


def _biases_zero(inputs):
    names = ['b_conv1', 'b_conv2', 'fc0_b', 'fc0c_b', 'fc1c_b', 'red_b',
             'g_self_b', 'g_rel_b', 'g_aff_b', 'g_out_b', 'agg_b', 'dec_b']
    return all(not np.any(np.asarray(inputs[n])) for n in names)


def kernel(**inputs):
    inputs = {k: np.asarray(v) for k, v in inputs.items()}
    nc = _get_nc(zero_bias=_biases_zero(inputs))
    maps = [make_core_inputs(inputs, s) for s in range(NCORE)]
    res = run_bass_kernel_spmd(nc, maps, core_ids=list(range(NCORE)))
    out = np.concatenate([res.results[s]["bbox_out"] for s in range(NCORE)], 0)
    return out.astype(np.float32)
